# revision 4
# baseline (speedup 1.0000x reference)
"""Trainium2 Bass kernel for causal self-attention with PoPE (v2, fp8).

Reference (B=2, T=2048, C=1024, H=16, D=64):
  qkv = x @ w_attn.T ; mu = softplus(q|k)
  q_aug = mu_q * [cos(tw), sin(tw)] ; k_aug = mu_k * [cos(tw+d), sin(tw+d)]
  att = softmax_causal((q_aug . k_aug)/8) ; out = (att @ v) @ w_proj.T

Sharding: 8 cores = 2 batches x 4 head-groups (4 heads each); host sums
the 4 c_proj partials per batch.

Design (fp8 DoubleRow matmuls at 0.5 cyc/col with K=256/pass; ACT
exp is the bottleneck engine at ~0.833 ns/col):
  - qk proj: fp8e4m3 DR, x_hi and 64*w quantized host-side
  - v  proj: fp8 DR 3-term (x_hi@wv_hi + x_hi@wv_lo + x_lo@wv_hi)
  - softplus: Exp(scale=1/64) psum->mu bf16, then Ln(bias=1) in place;
    all Exps before all Lns so the ACT table loads only 3x total
  - Qt: hi+lo fp8 residual pair in the DR j-dim; Kt: hi and lo each
    replicated across j via stride-0 APs => S = Qt.(Kt_hi+Kt_lo) exact
    to fp8-pair precision in 2 DR matmuls per 128-key tile
  - causal mask folded into the S psum as -240-additive fp8 matmuls
    (lower-strict const x split-identity), so exp gives 0; no mask ops
  - P = exp(S/8 - 2.5) -> bf16; y = P^T @ V_aug bf16 per 128-tq tile
    => y [tq, 64ch|denom] in psum; per-partition normalize (DVE recip +
    broadcast mul); PE transpose-mode -> yT; c_proj in bf16
  - tq blocks processed high-to-low so the large jb=3 exp workload
    covers Qt/Kt formation; heads software-pipelined (S/exp of head h+1
    emitted before y of head h); c_proj of block jb deferred into block
    jb-1's first head (PE queues are in-order)
  - gpsimd never touches PSUM (ISA restriction); residuals via
    tensor_sub; fp8/bf16 conversions split across DVE/Pool
Engine busy (CoreSim): ACT ~97us (bottleneck), PE ~83us, span ~130us.
End-to-end rel err ~1.07e-2 (budget 2e-2): dominant terms are single-
level fp8 x/w in the qk projection (~1.0%) and fp8 Kt (~0.3% after the
two-sided split).
"""

import math
import os
import sys

import numpy as np
import ml_dtypes

for _p in ("/opt/trn_rl_repo",):
    if _p not in sys.path and os.path.isdir(_p):
        sys.path.insert(0, _p)

import concourse.tile as tile
from concourse import bacc
from concourse import mybir
from concourse import bass_utils

B, T, C = 2, 2048, 1024
H, D = 16, 64
BASE = 10000.0
N_CORES = 8
HPC = 4  # heads per core
NTT = 16  # t tiles of 128

F32 = mybir.dt.float32
F32R = mybir.dt.float32r
BF16 = mybir.dt.bfloat16
FP8 = mybir.dt.float8e4
AF = mybir.ActivationFunctionType
PM = mybir.MatmulPerfMode
OP = mybir.AluOpType
E4 = ml_dtypes.float8_e4m3
WSCALE = 64.0


def build_module():
    nc = bacc.Bacc(
        "TRN2", target_bir_lowering=False, debug=False, num_devices=N_CORES
    )

    xhi_d = nc.dram_tensor("xhi", (4, 128, 2, T), FP8, kind="ExternalInput").ap()
    xlo_d = nc.dram_tensor("xlo", (4, 128, 2, T), FP8, kind="ExternalInput").ap()
    wqk_d = nc.dram_tensor("wqk", (4, 128, 2, 512), FP8, kind="ExternalInput").ap()
    wvh_d = nc.dram_tensor("wvh", (4, 128, 2, 256), FP8, kind="ExternalInput").ap()
    wvl_d = nc.dram_tensor("wvl", (4, 128, 2, 256), FP8, kind="ExternalInput").ap()
    trig_d = nc.dram_tensor("trig", (128, T), BF16, kind="ExternalInput").ap()
    ab_d = nc.dram_tensor("ab", (HPC, 128, T), BF16, kind="ExternalInput").ap()
    mlow_d = nc.dram_tensor("mlow", (64, 2, 128), FP8, kind="ExternalInput").ap()
    mful_d = nc.dram_tensor("mful", (64, 2, 128), FP8, kind="ExternalInput").ap()
    idsp_d = nc.dram_tensor("idsp", (64, 2, 128), FP8, kind="ExternalInput").ap()
    i128_d = nc.dram_tensor("i128", (128, 128), BF16, kind="ExternalInput").ap()
    w2_d = nc.dram_tensor("w2", (2, 128, 1024), BF16, kind="ExternalInput").ap()
    out_d = nc.dram_tensor("out", (NTT, 128, 1024), F32, kind="ExternalOutput").ap()

    with tile.TileContext(nc) as tc:
        with (
            tc.tile_pool(name="persist", bufs=1) as persist,
            tc.tile_pool(name="qkpool", bufs=1) as qkpool,
            tc.tile_pool(name="ps2", bufs=2, space="PSUM") as ps2,
            tc.tile_pool(name="psy", bufs=2, space="PSUM") as psy,
            tc.tile_pool(name="psc", bufs=2, space="PSUM") as psc,
        ):
            # ---- persistent constants / outputs-in-sbuf ----
            mlow = persist.tile([64, 2, 128], FP8)
            mful = persist.tile([64, 2, 128], FP8)
            idsp = persist.tile([64, 2, 128], FP8)
            i128 = persist.tile([128, 128], BF16)
            trig = persist.tile([128, T], BF16)
            c_inv64 = persist.tile([128, 1], F32)
            c_inv8 = persist.tile([128, 1], F32)
            c_bias = persist.tile([128, 1], F32)
            nc.gpsimd.memset(c_inv64, 1.0 / WSCALE)
            nc.gpsimd.memset(c_inv8, 0.125)
            nc.gpsimd.memset(c_bias, -2.5)
            v_aug = persist.tile([128, NTT, HPC, 65], BF16)
            nc.vector.memset(
                v_aug.rearrange("p a b c -> p (a b) c")[:, :, 64:65], 1.0
            )
            # per-head S inputs (fp8): qt = [hi|lo] in DR j-dim; kt single
            qts = [qkpool.tile([128, 2, T], FP8, name=f"qt{h}") for h in range(HPC)]
            kts = [qkpool.tile([128, T], FP8, name=f"kt{h}") for h in range(HPC)]
            ktlos = [qkpool.tile([128, T], FP8, name=f"ktlo{h}") for h in range(HPC)]
            y_t = persist.tile([128, NTT, HPC, 64], BF16)
            yT = persist.tile([128, 2, T], BF16)
            w2 = persist.tile([128, 2, 1024], BF16)
            
            from concourse.tile_rust import add_dep_helper
            act_groups = {"E": [], "L": []}
            # ---------------- Phase A: QKV projection ----------------
            mupool_ctx = tc.tile_pool(name="mupool", bufs=1)
            mupool = mupool_ctx.__enter__()
            mu = [mupool.tile([128, T], BF16, name=f"mu{h}") for h in range(HPC)]

            a5_ctx = tc.tile_pool(name="a5", bufs=2)
            a5 = a5_ctx.__enter__()
            abp_ctx = tc.tile_pool(name="abp", bufs=2)
            abp = abp_ctx.__enter__()
            phb_ctx = tc.tile_pool(name="phB", bufs=17)
            phb = phb_ctx.__enter__()
            ost_ctx = tc.tile_pool(name="ostage", bufs=3)
            ostage = ost_ctx.__enter__()
            rn_ctx = tc.tile_pool(name="rn", bufs=2)
            rnp = rn_ctx.__enter__()
            pha_ctx = tc.tile_pool(name="phA", bufs=1)
            pha = pha_ctx.__enter__()
            xhi = pha.tile([128, 4, 2, T], FP8)
            xlo = pha.tile([128, 4, 2, T], FP8)
            wqk = pha.tile([128, 4, 2, 512], FP8)
            wvh = pha.tile([128, 4, 2, 256], FP8)
            wvl = pha.tile([128, 4, 2, 256], FP8)
            nc.sync.dma_start(wqk, wqk_d.rearrange("o p j e -> p o j e"))
            engs = [nc.sync, nc.gpsimd, nc.sync, nc.gpsimd]
            for th in range(2):
                tsl = slice(th * 1024, th * 1024 + 1024)
                for cc in range(4):
                    engs[cc].dma_start(xhi[:, cc, :, tsl], xhi_d[cc][:, :, tsl])
            nc.sync.dma_start(wvh, wvh_d.rearrange("o p j e -> p o j e"))
            nc.sync.dma_start(wvl, wvl_d.rearrange("o p j e -> p o j e"))
            for th in range(2):
                tsl = slice(th * 1024, th * 1024 + 1024)
                for cc in range(4):
                    engs[(cc + 1) % 4].dma_start(
                        xlo[:, cc, :, tsl], xlo_d[cc][:, :, tsl]
                    )
            nc.gpsimd.dma_start(trig, trig_d)
            nc.gpsimd.dma_start(mlow, mlow_d)
            nc.gpsimd.dma_start(mful, mful_d)
            nc.gpsimd.dma_start(idsp, idsp_d)
            nc.gpsimd.dma_start(i128, i128_d)
            nc.gpsimd.dma_start(w2, w2_d.rearrange("o p e -> p o e"))

            # qk: per (head, 1024-block): psum [128,1024], 4 DR matmuls over cc
            # v: out[t, 256] per t-tile, 4 tiles per psum; 3-term fp8 DR.
            # Interleave qk and v allocations so PE has v work while ACT
            # drains softplus.
            def emit_qk(h, tb):
                ts_ = slice(tb * 1024, tb * 1024 + 1024)
                qk_ps = ps2.tile([128, 1024], F32, tag="s2", name="qk_ps")
                for half in range(2):
                    hs = slice(tb * 1024 + half * 512, tb * 1024 + half * 512 + 512)
                    for cc in range(4):
                        nc.tensor.matmul(
                            qk_ps[:, half * 512 : half * 512 + 512],
                            lhsT=wqk[:, cc, :, h * 128 : (h + 1) * 128],
                            rhs=xhi[:, cc, :, hs],
                            start=(cc == 0),
                            stop=(cc == 3),
                            perf_mode=PM.DoubleRow,
                        )
                # softplus = ln(exp(q/64)+1): exp -> mu (bf16), ln batched
                # later so the ACT table doesn't ping-pong between sets
                act_groups["E"].append(
                    nc.scalar.activation(mu[h][:, ts_], qk_ps, AF.Exp, scale=c_inv64)
                )

            def emit_v(tq):  # tq = t-pair index 0..7
                v_ps = psc.tile([128, 2, 256], F32, tag="pc", name="v_ps")
                for s in range(2):
                    tt = 2 * tq + s
                    tsl = slice(tt * 128, (tt + 1) * 128)
                    terms = ((xhi, wvh), (xhi, wvl), (xlo, wvh))
                    n = 0
                    for cc in range(4):
                        for (xx, ww) in terms:
                            nc.tensor.matmul(
                                v_ps[:, s, :],
                                lhsT=xx[:, cc, :, tsl],
                                rhs=ww[:, cc],
                                start=(n == 0),
                                stop=(n == 11),
                                perf_mode=PM.DoubleRow,
                            )
                            n += 1
                nc.vector.tensor_copy(
                    out=v_aug[:, 2 * tq : 2 * tq + 2, :, 0:64],
                    in_=v_ps.rearrange("p s (h e) -> p s h e", h=HPC),
                )

            for h in range(HPC):
                emit_qk(h, 0)
                emit_qk(h, 1)
            for tq in range(8):
                emit_v(tq)
            for h in range(HPC):
                ln = nc.scalar.activation(mu[h], mu[h], AF.Ln, bias=1.0)
                add_dep_helper(ln.ins, act_groups["E"][-1].ins, sync=False,
                               reason="group phase-A Lns after Exps (ACT table)")
                act_groups["L"].append(ln)

            pha_ctx.__exit__(None, None, None)

            # ------- Phase A.5: Qt/Kt formation (per head) -------
            for h in range(HPC):
                abh = abp.tile([128, T], BF16, tag="ab", name="abh")
                nc.sync.dma_start(abh, ab_d[h])
                # musw = [mu_k ; mu_q] (swapped halves) via DVE shuffles;
                # processed in 1024-col halves, high half first (jb runs
                # descending, so high-t qt cols are needed first; kt pair 0
                # needs low-t keys first -> kt low half first)
                musw = a5.tile([128, T], BF16, tag="musw", name="musw")
                qtf = a5.tile([128, T], BF16, tag="qtf", name="qtf")
                ktf = a5.tile([128, T], BF16, tag="ktf", name="ktf")
                idm = list(range(32))
                qeng = nc.vector if h == 0 else nc.gpsimd
                for hb in (0,) if h == 0 else ((1, 0)):
                    # kt chain on Pool (low keys first: S pair 0 reads them)
                    ts_ = slice(hb * 1024, hb * 1024 + 1024)
                    nc.vector.stream_shuffle(
                        musw[0:64, ts_], mu[h][64:128, ts_], idm
                    )
                    nc.gpsimd.tensor_mul(
                        ktf[0:64, ts_], musw[0:64, ts_], abh[0:64, ts_]
                    )
                    nc.gpsimd.tensor_mul(
                        ktf[64:128, ts_], mu[h][64:128, ts_], abh[64:128, ts_]
                    )
                    nc.gpsimd.tensor_copy(out=kts[h][:, ts_], in_=ktf[:, ts_])
                    nc.gpsimd.tensor_sub(
                        ktlos[h][:, ts_], ktf[:, ts_], kts[h][:, ts_]
                    )
                for hb in ((1, 0) if h != 0 else (1,)):
                    # qt chain on DVE (h0: incl. fp8 ops, parallel with Pool)
                    ts_ = slice(hb * 1024, hb * 1024 + 1024)
                    nc.vector.stream_shuffle(
                        musw[64:128, ts_], mu[h][0:64, ts_], idm
                    )
                    nc.vector.tensor_mul(
                        qtf[0:64, ts_], mu[h][0:64, ts_], trig[0:64, ts_]
                    )
                    nc.vector.tensor_mul(
                        qtf[64:128, ts_], musw[64:128, ts_], trig[64:128, ts_]
                    )
                    qeng.tensor_copy(
                        out=qts[h][:, 0, ts_], in_=qtf[:, ts_]
                    )
                    qeng.tensor_sub(
                        qts[h][:, 1, ts_], qtf[:, ts_], qts[h][:, 0, ts_]
                    )
                if h == 0:
                    for ts_ in (slice(1024, 2048), slice(0, 1024)):
                        nc.vector.stream_shuffle(
                            musw[0:64, ts_] if ts_.start == 1024 else musw[64:128, ts_],
                            mu[h][64:128, ts_] if ts_.start == 1024 else mu[h][0:64, ts_],
                            idm,
                        )
                    ts_ = slice(1024, 2048)
                    nc.gpsimd.tensor_mul(
                        ktf[0:64, ts_], musw[0:64, ts_], abh[0:64, ts_]
                    )
                    nc.gpsimd.tensor_mul(
                        ktf[64:128, ts_], mu[h][64:128, ts_], abh[64:128, ts_]
                    )
                    nc.gpsimd.tensor_copy(out=kts[h][:, ts_], in_=ktf[:, ts_])
                    nc.gpsimd.tensor_sub(
                        ktlos[h][:, ts_], ktf[:, ts_], kts[h][:, ts_]
                    )
                    ts_ = slice(0, 1024)
                    nc.vector.tensor_mul(
                        qtf[0:64, ts_], mu[h][0:64, ts_], trig[0:64, ts_]
                    )
                    nc.vector.tensor_mul(
                        qtf[64:128, ts_], musw[64:128, ts_], trig[64:128, ts_]
                    )
                    nc.vector.tensor_copy(out=qts[h][:, 0, ts_], in_=qtf[:, ts_])
                    nc.vector.tensor_sub(
                        qts[h][:, 1, ts_], qtf[:, ts_], qts[h][:, 0, ts_]
                    )

            # ------- Phase B: attention + transpose + c_proj, per tq block ----
            proj_pending = []
            emit_proj_fns = {}
            for jb in (3, 2, 1, 0):
                cols = slice(jb * 512, (jb + 1) * 512)
                def emit_y(h, p_sbs, y_ps):
                    for q in range(4):
                        jt = 4 * jb + q
                        for kt in range(jt + 1):
                            nc.tensor.matmul(
                                y_ps[:, q, :],
                                lhsT=p_sbs[kt // 2][:, kt % 2, q * 128 : (q + 1) * 128],
                                rhs=v_aug[:, kt, h, :],
                                start=(kt == 0),
                                stop=(kt == jt),
                            )
                    rn = rnp.tile([128, 4], F32, tag="rn", name="rn")
                    nc.vector.reciprocal(
                        rn, y_ps[:, :, 64:65].rearrange("p a b -> p (a b)")
                    )
                    rnb = rn.rearrange("p (a b) -> p a b", b=1).broadcast_to(
                        [128, 4, 64]
                    )
                    nc.vector.tensor_mul(
                        y_t[:, 4 * jb : 4 * jb + 4, h, :], y_ps[:, :, 0:64], rnb
                    )

                pending = None
                for h in range(HPC):
                    y_ps = psy.tile([128, 4, 65], F32, tag="y1", name="y_ps")
                    p_sbs = []
                    for p in range(2 * jb + 2):
                        c0 = max(0, 256 * p - 512 * jb)
                        csl = slice(jb * 512 + c0, (jb + 1) * 512)
                        sps = ps2.tile([128, 2, 512], F32, tag="s2", name="sps")
                        for s in range(2):
                            kt = 2 * p + s
                            ksl = slice(kt * 128, (kt + 1) * 128)
                            ktb = kts[h][:, ksl].rearrange(
                                "p (j m) -> p j m", j=1
                            ).broadcast_to([128, 2, 128])
                            ktlob = ktlos[h][:, ksl].rearrange(
                                "p (j m) -> p j m", j=1
                            ).broadcast_to([128, 2, 128])
                            nc.tensor.matmul(
                                sps[:, s, c0:512],
                                lhsT=ktb,
                                rhs=qts[h][:, :, csl],
                                start=True,
                                stop=False,
                                perf_mode=PM.DoubleRow,
                            )
                            d0 = 128 * kt - 512 * jb  # diag block col offset
                            if 0 <= d0 < 512:
                                nc.tensor.matmul(
                                    sps[:, s, d0 : d0 + 128],
                                    lhsT=mlow,
                                    rhs=idsp,
                                    start=False,
                                    stop=False,
                                    perf_mode=PM.DoubleRow,
                                )
                                if d0 > c0:
                                    # fully-masked strip left of the diagonal
                                    nc.tensor.matmul(
                                        sps[:, s, c0:d0],
                                        lhsT=mful,
                                        rhs=idsp[:, :, 0 : d0 - c0],
                                        start=False,
                                        stop=False,
                                        perf_mode=PM.DoubleRow,
                                    )
                            nc.tensor.matmul(
                                sps[:, s, c0:512],
                                lhsT=ktlob,
                                rhs=qts[h][:, :, csl],
                                start=False,
                                stop=True,
                                perf_mode=PM.DoubleRow,
                            )
                        p_sb = phb.tile([128, 2, 512], BF16, tag="p", name="p_sb")
                        ex = nc.scalar.activation(
                            p_sb[:, :, c0:512],
                            sps[:, :, c0:512],
                            AF.Exp,
                            scale=c_inv8,
                            bias=c_bias,
                        )
                        if act_groups["L"]:
                            add_dep_helper(ex.ins, act_groups["L"][-1].ins, sync=False,
                                           reason="phase-B exps after phase-A Lns (ACT table)")
                            act_groups["L"] = []
                        p_sbs.append(p_sb)
                    if h == 0 and proj_pending:
                        emit_proj_fns[proj_pending.pop()]()
                    if pending is not None:
                        emit_y(*pending)
                    pending = (h, p_sbs, y_ps)
                emit_y(*pending)
                # transpose 4 t-tiles + c_proj + stage + dma out (deferred:
                # emitted after the next jb's first head S/exp so PE's
                # in-order queue doesn't stall ACT at jb boundaries)
                def emit_proj(jb=jb, last=False):
                  for q in range(4):
                    tt = 4 * jb + q
                    tsl = slice(tt * 128, (tt + 1) * 128)
                    pc = psc.tile([128, 2, 128], BF16, tag="pc", name="pc")
                    for cc in range(2):
                        nc.tensor.matmul(
                            pc[:, cc, :],
                            lhsT=y_t[:, tt, 2 * cc : 2 * cc + 2, :],
                            rhs=i128,
                            start=True,
                            stop=True,
                            is_transpose=True,
                        )
                    nc.vector.tensor_copy(out=yT[:, :, tsl], in_=pc)
                  for q in range(4):
                    tt = 4 * jb + q
                    tsl = slice(tt * 128, (tt + 1) * 128)
                    ost = ostage.tile([128, 1024], F32, tag="o", name="ost")
                    for eh in range(2):
                        po = psc.tile([128, 512], F32, tag="pc", name="po")
                        for cc in range(2):
                            nc.tensor.matmul(
                                po,
                                lhsT=yT[:, cc, tsl],
                                rhs=w2[:, cc, eh * 512 : (eh + 1) * 512],
                                start=(cc == 0),
                                stop=(cc == 1),
                            )
                        if last and (2 * q + eh) % 2 == 1:
                            nc.scalar.copy(ost[:, eh * 512 : (eh + 1) * 512], po)
                        else:
                            nc.vector.tensor_copy(
                                out=ost[:, eh * 512 : (eh + 1) * 512], in_=po
                            )
                    deng = nc.sync if q % 2 == 0 else nc.gpsimd
                    deng.dma_start(out_d[tt], ost)
                emit_proj_fns[jb] = emit_proj
                proj_pending.append(jb)
            while proj_pending:
                emit_proj_fns[proj_pending.pop()](last=True)
            rn_ctx.__exit__(None, None, None)
            ost_ctx.__exit__(None, None, None)
            phb_ctx.__exit__(None, None, None)
            abp_ctx.__exit__(None, None, None)
            a5_ctx.__exit__(None, None, None)
            mupool_ctx.__exit__(None, None, None)

    nc.compile()
    return nc


def make_inputs(x, w_attn, w_proj, delta):
    """Host-side prep: per-core input dicts (core = b*4 + g)."""
    x = np.asarray(x, dtype=np.float32)
    w_attn = np.asarray(w_attn, dtype=np.float32)
    w_proj = np.asarray(w_proj, dtype=np.float32)
    delta = np.asarray(delta, dtype=np.float32)

    inv_freq = 1.0 / (BASE ** (np.arange(D, dtype=np.float32) / D))
    t = np.arange(T, dtype=np.float64)
    freqs = (t[:, None] * inv_freq[None, :].astype(np.float64)).astype(np.float32)
    cosT = np.cos(freqs).T.astype(np.float32)  # (D, T)
    sinT = np.sin(freqs).T.astype(np.float32)
    trig = np.concatenate([cosT, sinT], axis=0).astype(ml_dtypes.bfloat16)
    d = np.clip(delta, -2.0 * math.pi, 0.0)

    # fp8 split of x per batch: [4cc, 128p, 2j, T] with c = cc*256+j*128+p
    def to_dr(mat):  # mat (T, C) -> (4, 128, 2, T)
        m = mat.T.reshape(4, 2, 128, T)  # (cc, j, p, t)
        return np.ascontiguousarray(m.transpose(0, 2, 1, 3))

    xw = [None] * B
    for b in range(B):
        xb = x[b]  # (T, C)
        x_hi = xb.astype(E4)
        x_lo = (xb - x_hi.astype(np.float32)).astype(E4)
        xw[b] = (to_dr(x_hi.astype(np.float32)).astype(E4),
                 to_dr(x_lo.astype(np.float32)).astype(E4))

    qw = w_attn[:C].reshape(H, D, C)
    kw = w_attn[C : 2 * C].reshape(H, D, C)
    vw = w_attn[2 * C :].reshape(H, D, C)

    # mask constants
    pp, jj, tk = np.meshgrid(
        np.arange(64), np.arange(2), np.arange(128), indexing="ij"
    )
    f = jj * 64 + pp
    mlow = np.where(tk > f, -240.0, 0.0).astype(E4)
    mful = np.full((64, 2, 128), -240.0, dtype=np.float32).astype(E4)
    idsp = (tk == f).astype(np.float32).astype(E4)
    i128 = np.eye(128, dtype=np.float32).astype(ml_dtypes.bfloat16)

    in_maps = []
    for core in range(N_CORES):
        b, g = divmod(core, HPC)
        heads = list(range(HPC * g, HPC * g + HPC))

        # wqk: (4cc, 128p, 2j, 512): col = h*128 + r; r<64 q_d else k_d
        wqk_full = np.empty((C, 512), dtype=np.float32)  # (c, col)
        for hi_, hg in enumerate(heads):
            wqk_full[:, hi_ * 128 : hi_ * 128 + 64] = qw[hg].T * WSCALE
            wqk_full[:, hi_ * 128 + 64 : hi_ * 128 + 128] = kw[hg].T * WSCALE
        wqk8 = wqk_full.astype(E4)
        wqk_dr = np.ascontiguousarray(
            wqk8.reshape(4, 2, 128, 512).transpose(0, 2, 1, 3)
        )

        wv_full = (
            vw[HPC * g : HPC * g + HPC].reshape(256, C).T * WSCALE
        )  # (c, 256)
        wv_hi = wv_full.astype(E4)
        wv_lo = (wv_full - wv_hi.astype(np.float32)).astype(E4)
        wvh_dr = np.ascontiguousarray(
            wv_hi.reshape(4, 2, 128, 256).transpose(0, 2, 1, 3)
        )
        wvl_dr = np.ascontiguousarray(
            wv_lo.reshape(4, 2, 128, 256).transpose(0, 2, 1, 3)
        )

        ab = np.stack(
            [
                np.concatenate(
                    [
                        np.cos(freqs + d[hg][None, :]).T,
                        np.sin(freqs + d[hg][None, :]).T,
                    ],
                    axis=0,
                ).astype(ml_dtypes.bfloat16)
                for hg in heads
            ],
            axis=0,
        )  # (4, 128, T)

        # w2: (2cc, 128p, 1024e): channel c_local = cc*128 + p of this group's
        # 256 y channels; y channel (h_local, dd) flattened h_local*64+dd
        w2g = w_proj[:, 256 * g : 256 * (g + 1)]  # (e, 256)
        w2_dr = np.ascontiguousarray((w2g.T / WSCALE).reshape(2, 128, 1024)).astype(ml_dtypes.bfloat16)

        in_maps.append(
            {
                "xhi": xw[b][0],
                "xlo": xw[b][1],
                "wqk": wqk_dr,
                "wvh": wvh_dr,
                "wvl": wvl_dr,
                "trig": trig,
                "ab": ab,
                "mlow": mlow,
                "mful": mful,
                "idsp": idsp,
                "i128": i128,
                "w2": w2_dr,
            }
        )
    return in_maps


_NC_CACHE = []


def _get_nc():
    if not _NC_CACHE:
        _NC_CACHE.append(build_module())
    return _NC_CACHE[0]


def kernel(x, w_attn, w_proj, delta, _trace=False):
    in_maps = make_inputs(x, w_attn, w_proj, delta)
    nc = _get_nc()
    res = None
    outs = None
    last_err = None
    for attempt in range(3):
        try:
            res = bass_utils.run_bass_kernel_spmd(
                nc, in_maps, core_ids=list(range(N_CORES)), trace=_trace
            )
            outs = [np.asarray(r["out"]).reshape(T, C) for r in res.results]
            break
        except Exception as e:
            last_err = e
            if "unrecoverable" not in str(e).lower() or attempt == 2:
                raise
            import time as _time

            _time.sleep(2.0)
    assert outs is not None, last_err
    if _trace:
        kernel.last_results = res
    full = np.zeros((B, T, C), dtype=np.float32)
    for core in range(N_CORES):
        full[core // HPC] += outs[core]
    return full


# revision 5
# speedup vs baseline: 1.0499x; 1.0499x over previous
"""Trainium2 Bass kernel for causal self-attention with PoPE (v2, fp8).

Reference (B=2, T=2048, C=1024, H=16, D=64):
  qkv = x @ w_attn.T ; mu = softplus(q|k)
  q_aug = mu_q * [cos(tw), sin(tw)] ; k_aug = mu_k * [cos(tw+d), sin(tw+d)]
  att = softmax_causal((q_aug . k_aug)/8) ; out = (att @ v) @ w_proj.T

Sharding: 8 cores = 2 batches x 4 head-groups (4 heads each); host sums
the 4 c_proj partials per batch.

Design (fp8 DoubleRow matmuls at 0.5 cyc/col with K=256/pass; ACT
exp is the bottleneck engine at ~0.833 ns/col):
  - qk proj: fp8e4m3 DR, x_hi and 64*w quantized host-side
  - v  proj: fp8 DR 3-term (x_hi@wv_hi + x_hi@wv_lo + x_lo@wv_hi)
  - softplus: Exp(scale=1/64) psum->mu bf16, then Ln(bias=1) in place;
    all Exps before all Lns so the ACT table loads only 3x total
  - Qt: hi+lo fp8 residual pair in the DR j-dim; Kt: hi and lo each
    replicated across j via stride-0 APs => S = Qt.(Kt_hi+Kt_lo) exact
    to fp8-pair precision in 2 DR matmuls per 128-key tile
  - causal mask folded into the S psum as -240-additive fp8 matmuls
    (lower-strict const x split-identity), so exp gives 0; no mask ops
  - P = exp(S/8 - 2.5) -> bf16; y = P^T @ V_aug bf16 per 128-tq tile
    => y [tq, 64ch|denom] in psum; per-partition normalize (DVE recip +
    broadcast mul); PE transpose-mode -> yT; c_proj in bf16
  - tq blocks processed high-to-low so the large jb=3 exp workload
    covers Qt/Kt formation; heads software-pipelined (S/exp of head h+1
    emitted before y of head h); c_proj of block jb deferred into block
    jb-1's first head (PE queues are in-order)
  - gpsimd never touches PSUM (ISA restriction); residuals via
    tensor_sub; fp8/bf16 conversions split across DVE/Pool
Engine busy (CoreSim): ACT ~97us (bottleneck), PE ~83us, span ~130us.
End-to-end rel err ~1.07e-2 (budget 2e-2): dominant terms are single-
level fp8 x/w in the qk projection (~1.0%) and fp8 Kt (~0.3% after the
two-sided split).
"""

import math
import os
import sys

import numpy as np
import ml_dtypes

for _p in ("/opt/trn_rl_repo",):
    if _p not in sys.path and os.path.isdir(_p):
        sys.path.insert(0, _p)

import concourse.tile as tile
from concourse import bacc
from concourse import mybir
from concourse import bass_utils

B, T, C = 2, 2048, 1024
H, D = 16, 64
BASE = 10000.0
N_CORES = 8
HPC = 4  # heads per core
NTT = 16  # t tiles of 128

F32 = mybir.dt.float32
F32R = mybir.dt.float32r
BF16 = mybir.dt.bfloat16
FP8 = mybir.dt.float8e4
AF = mybir.ActivationFunctionType
PM = mybir.MatmulPerfMode
OP = mybir.AluOpType
E4 = ml_dtypes.float8_e4m3
WSCALE = 64.0


def build_module():
    nc = bacc.Bacc(
        "TRN2", target_bir_lowering=False, debug=False, num_devices=N_CORES
    )

    xhi_d = nc.dram_tensor("xhi", (4, 128, 2, T), FP8, kind="ExternalInput").ap()
    xlo_d = nc.dram_tensor("xlo", (4, 128, 2, T), FP8, kind="ExternalInput").ap()
    wqk_d = nc.dram_tensor("wqk", (4, 128, 2, 512), FP8, kind="ExternalInput").ap()
    wvh_d = nc.dram_tensor("wvh", (4, 128, 2, 256), FP8, kind="ExternalInput").ap()
    wvl_d = nc.dram_tensor("wvl", (4, 128, 2, 256), FP8, kind="ExternalInput").ap()
    trig_d = nc.dram_tensor("trig", (128, T), BF16, kind="ExternalInput").ap()
    ab_d = nc.dram_tensor("ab", (HPC, 128, T), BF16, kind="ExternalInput").ap()
    mlow_d = nc.dram_tensor("mlow", (64, 2, 128), FP8, kind="ExternalInput").ap()
    mful_d = nc.dram_tensor("mful", (64, 2, 128), FP8, kind="ExternalInput").ap()
    idsp_d = nc.dram_tensor("idsp", (64, 2, 128), FP8, kind="ExternalInput").ap()
    i128_d = nc.dram_tensor("i128", (128, 128), BF16, kind="ExternalInput").ap()
    w2_d = nc.dram_tensor("w2", (2, 128, 1024), BF16, kind="ExternalInput").ap()
    out_d = nc.dram_tensor("out", (NTT, 128, 1024), F32, kind="ExternalOutput").ap()

    with tile.TileContext(nc) as tc:
        with (
            tc.tile_pool(name="persist", bufs=1) as persist,
            tc.tile_pool(name="qkpool", bufs=1) as qkpool,
            tc.tile_pool(name="ps2", bufs=2, space="PSUM") as ps2,
            tc.tile_pool(name="psy", bufs=2, space="PSUM") as psy,
            tc.tile_pool(name="psc", bufs=2, space="PSUM") as psc,
        ):
            # ---- persistent constants / outputs-in-sbuf ----
            mlow = persist.tile([64, 2, 128], FP8)
            mful = persist.tile([64, 2, 128], FP8)
            idsp = persist.tile([64, 2, 128], FP8)
            i128 = persist.tile([128, 128], BF16)
            trig = persist.tile([128, T], BF16)
            c_inv64 = persist.tile([128, 1], F32)
            c_inv8 = persist.tile([128, 1], F32)
            c_bias = persist.tile([128, 1], F32)
            nc.gpsimd.memset(c_inv64, 1.0 / WSCALE)
            nc.gpsimd.memset(c_inv8, 0.125)
            nc.gpsimd.memset(c_bias, -2.5)
            v_aug = persist.tile([128, NTT, HPC, 65], BF16)
            nc.vector.memset(
                v_aug.rearrange("p a b c -> p (a b) c")[:, :, 64:65], 1.0
            )
            # per-head S inputs (fp8): qt = [hi|lo] in DR j-dim; kt single
            qts = [qkpool.tile([128, 2, T], FP8, name=f"qt{h}") for h in range(HPC)]
            kts = [qkpool.tile([128, T], FP8, name=f"kt{h}") for h in range(HPC)]
            ktlos = [qkpool.tile([128, T], FP8, name=f"ktlo{h}") for h in range(HPC)]
            y_t = persist.tile([128, NTT, HPC, 64], BF16)
            yT = persist.tile([128, 2, T], BF16)
            w2 = persist.tile([128, 2, 1024], BF16)
            
            from concourse.tile_rust import add_dep_helper
            act_groups = {"E": [], "L": []}
            # ---------------- Phase A: QKV projection ----------------
            mupool_ctx = tc.tile_pool(name="mupool", bufs=1)
            mupool = mupool_ctx.__enter__()
            mu = [mupool.tile([128, T], BF16, name=f"mu{h}") for h in range(HPC)]

            a5_ctx = tc.tile_pool(name="a5", bufs=2)
            a5 = a5_ctx.__enter__()
            abp_ctx = tc.tile_pool(name="abp", bufs=2)
            abp = abp_ctx.__enter__()
            phb_ctx = tc.tile_pool(name="phB", bufs=17)
            phb = phb_ctx.__enter__()
            ost_ctx = tc.tile_pool(name="ostage", bufs=3)
            ostage = ost_ctx.__enter__()
            rn_ctx = tc.tile_pool(name="rn", bufs=2)
            rnp = rn_ctx.__enter__()
            pha_ctx = tc.tile_pool(name="phA", bufs=1)
            pha = pha_ctx.__enter__()
            xhi = pha.tile([128, 4, 2, T], FP8)
            xlo = pha.tile([128, 4, 2, T], FP8)
            wqk = pha.tile([128, 4, 2, 512], FP8)
            wvh = pha.tile([128, 4, 2, 256], FP8)
            wvl = pha.tile([128, 4, 2, 256], FP8)
            nc.sync.dma_start(wqk, wqk_d.rearrange("o p j e -> p o j e"))
            engs = [nc.sync, nc.gpsimd, nc.sync, nc.gpsimd]
            for th in range(2):
                tsl = slice(th * 1024, th * 1024 + 1024)
                for cc in range(4):
                    engs[cc].dma_start(xhi[:, cc, :, tsl], xhi_d[cc][:, :, tsl])
            nc.sync.dma_start(wvh, wvh_d.rearrange("o p j e -> p o j e"))
            nc.sync.dma_start(wvl, wvl_d.rearrange("o p j e -> p o j e"))
            for th in range(2):
                tsl = slice(th * 1024, th * 1024 + 1024)
                for cc in range(4):
                    engs[(cc + 1) % 4].dma_start(
                        xlo[:, cc, :, tsl], xlo_d[cc][:, :, tsl]
                    )
            nc.gpsimd.dma_start(trig, trig_d)
            nc.gpsimd.dma_start(mlow, mlow_d)
            nc.gpsimd.dma_start(mful, mful_d)
            nc.gpsimd.dma_start(idsp, idsp_d)
            nc.gpsimd.dma_start(i128, i128_d)
            nc.gpsimd.dma_start(w2, w2_d.rearrange("o p e -> p o e"))

            # qk: per (head, 1024-block): psum [128,1024], 4 DR matmuls over cc
            # v: out[t, 256] per t-tile, 4 tiles per psum; 3-term fp8 DR.
            # Interleave qk and v allocations so PE has v work while ACT
            # drains softplus.
            def emit_qk(h, tb):
                ts_ = slice(tb * 1024, tb * 1024 + 1024)
                qk_ps = ps2.tile([128, 1024], F32, tag="s2", name="qk_ps")
                for half in range(2):
                    hs = slice(tb * 1024 + half * 512, tb * 1024 + half * 512 + 512)
                    for cc in range(4):
                        nc.tensor.matmul(
                            qk_ps[:, half * 512 : half * 512 + 512],
                            lhsT=wqk[:, cc, :, h * 128 : (h + 1) * 128],
                            rhs=xhi[:, cc, :, hs],
                            start=(cc == 0),
                            stop=(cc == 3),
                            perf_mode=PM.DoubleRow,
                        )
                # softplus = ln(exp(q/64)+1): exp -> mu (bf16), ln batched
                # later so the ACT table doesn't ping-pong between sets
                act_groups["E"].append(
                    nc.scalar.activation(mu[h][:, ts_], qk_ps, AF.Exp, scale=c_inv64)
                )

            def emit_v(tq):  # tq = t-pair index 0..7
                v_ps = psc.tile([128, 2, 256], F32, tag="pc", name="v_ps")
                for s in range(2):
                    tt = 2 * tq + s
                    tsl = slice(tt * 128, (tt + 1) * 128)
                    terms = ((xhi, wvh), (xhi, wvl), (xlo, wvh))
                    n = 0
                    for cc in range(4):
                        for (xx, ww) in terms:
                            nc.tensor.matmul(
                                v_ps[:, s, :],
                                lhsT=xx[:, cc, :, tsl],
                                rhs=ww[:, cc],
                                start=(n == 0),
                                stop=(n == 11),
                                perf_mode=PM.DoubleRow,
                            )
                            n += 1
                nc.vector.tensor_copy(
                    out=v_aug[:, 2 * tq : 2 * tq + 2, :, 0:64],
                    in_=v_ps.rearrange("p s (h e) -> p s h e", h=HPC),
                )

            for h in range(HPC):
                emit_qk(h, 0)
                emit_qk(h, 1)
            for tq in range(8):
                emit_v(tq)
            for h in range(HPC):
                ln = nc.scalar.activation(mu[h], mu[h], AF.Ln, bias=1.0)
                add_dep_helper(ln.ins, act_groups["E"][-1].ins, sync=False,
                               reason="group phase-A Lns after Exps (ACT table)")
                act_groups["L"].append(ln)

            pha_ctx.__exit__(None, None, None)

            # ------- Phase A.5: Qt/Kt formation (per head) -------
            for h in range(HPC):
                abh = abp.tile([128, T], BF16, tag="ab", name="abh")
                nc.sync.dma_start(abh, ab_d[h])
                # musw = [mu_k ; mu_q] (swapped halves) via DVE shuffles;
                # processed in 1024-col halves, high half first (jb runs
                # descending, so high-t qt cols are needed first; kt pair 0
                # needs low-t keys first -> kt low half first)
                musw = a5.tile([128, T], BF16, tag="musw", name="musw")
                qtf = a5.tile([128, T], BF16, tag="qtf", name="qtf")
                ktf = a5.tile([128, T], BF16, tag="ktf", name="ktf")
                idm = list(range(32))
                qeng = nc.vector if h == 0 else nc.gpsimd
                for hb in (0,) if h == 0 else ((1, 0)):
                    # kt chain on Pool (low keys first: S pair 0 reads them)
                    ts_ = slice(hb * 1024, hb * 1024 + 1024)
                    if h == 0:
                        nc.vector.stream_shuffle(
                            musw[0:64, ts_], mu[h][64:128, ts_], idm
                        )
                    else:
                        nc.sync.dma_start(musw[0:64, ts_], mu[h][64:128, ts_])
                    nc.gpsimd.tensor_mul(
                        ktf[0:64, ts_], musw[0:64, ts_], abh[0:64, ts_]
                    )
                    nc.gpsimd.tensor_mul(
                        ktf[64:128, ts_], mu[h][64:128, ts_], abh[64:128, ts_]
                    )
                    nc.gpsimd.tensor_copy(out=kts[h][:, ts_], in_=ktf[:, ts_])
                    nc.gpsimd.tensor_sub(
                        ktlos[h][:, ts_], ktf[:, ts_], kts[h][:, ts_]
                    )
                for hb in ((1, 0) if h != 0 else (1,)):
                    # qt chain on DVE (h0: incl. fp8 ops, parallel with Pool)
                    ts_ = slice(hb * 1024, hb * 1024 + 1024)
                    if h == 0:
                        nc.vector.stream_shuffle(
                            musw[64:128, ts_], mu[h][0:64, ts_], idm
                        )
                    else:
                        nc.sync.dma_start(musw[64:128, ts_], mu[h][0:64, ts_])
                    nc.vector.tensor_mul(
                        qtf[0:64, ts_], mu[h][0:64, ts_], trig[0:64, ts_]
                    )
                    nc.vector.tensor_mul(
                        qtf[64:128, ts_], musw[64:128, ts_], trig[64:128, ts_]
                    )
                    qeng.tensor_copy(
                        out=qts[h][:, 0, ts_], in_=qtf[:, ts_]
                    )
                    qeng.tensor_sub(
                        qts[h][:, 1, ts_], qtf[:, ts_], qts[h][:, 0, ts_]
                    )
                if h == 0:
                    for ts_ in (slice(1024, 2048), slice(0, 1024)):
                        nc.vector.stream_shuffle(
                            musw[0:64, ts_] if ts_.start == 1024 else musw[64:128, ts_],
                            mu[h][64:128, ts_] if ts_.start == 1024 else mu[h][0:64, ts_],
                            idm,
                        )
                    ts_ = slice(1024, 2048)
                    nc.gpsimd.tensor_mul(
                        ktf[0:64, ts_], musw[0:64, ts_], abh[0:64, ts_]
                    )
                    nc.gpsimd.tensor_mul(
                        ktf[64:128, ts_], mu[h][64:128, ts_], abh[64:128, ts_]
                    )
                    nc.gpsimd.tensor_copy(out=kts[h][:, ts_], in_=ktf[:, ts_])
                    nc.gpsimd.tensor_sub(
                        ktlos[h][:, ts_], ktf[:, ts_], kts[h][:, ts_]
                    )
                    ts_ = slice(0, 1024)
                    nc.vector.tensor_mul(
                        qtf[0:64, ts_], mu[h][0:64, ts_], trig[0:64, ts_]
                    )
                    nc.vector.tensor_mul(
                        qtf[64:128, ts_], musw[64:128, ts_], trig[64:128, ts_]
                    )
                    nc.vector.tensor_copy(out=qts[h][:, 0, ts_], in_=qtf[:, ts_])
                    nc.vector.tensor_sub(
                        qts[h][:, 1, ts_], qtf[:, ts_], qts[h][:, 0, ts_]
                    )

            # ------- Phase B: attention + transpose + c_proj, per tq block ----
            proj_pending = []
            emit_proj_fns = {}
            for jb in (3, 2, 1, 0):
                cols = slice(jb * 512, (jb + 1) * 512)
                def emit_y(h, p_sbs, y_ps):
                    for q in range(4):
                        jt = 4 * jb + q
                        for kt in range(jt + 1):
                            nc.tensor.matmul(
                                y_ps[:, q, :],
                                lhsT=p_sbs[kt // 2][:, kt % 2, q * 128 : (q + 1) * 128],
                                rhs=v_aug[:, kt, h, :],
                                start=(kt == 0),
                                stop=(kt == jt),
                            )
                    rn = rnp.tile([128, 4], F32, tag="rn", name="rn")
                    nc.vector.reciprocal(
                        rn, y_ps[:, :, 64:65].rearrange("p a b -> p (a b)")
                    )
                    rnb = rn.rearrange("p (a b) -> p a b", b=1).broadcast_to(
                        [128, 4, 64]
                    )
                    nc.vector.tensor_mul(
                        y_t[:, 4 * jb : 4 * jb + 4, h, :], y_ps[:, :, 0:64], rnb
                    )

                pending = None
                for h in range(HPC):
                    y_ps = psy.tile([128, 4, 65], F32, tag="y1", name="y_ps")
                    p_sbs = []
                    for p in range(2 * jb + 2):
                        c0 = max(0, 256 * p - 512 * jb)
                        csl = slice(jb * 512 + c0, (jb + 1) * 512)
                        sps = ps2.tile([128, 2, 512], F32, tag="s2", name="sps")
                        for s in range(2):
                            kt = 2 * p + s
                            ksl = slice(kt * 128, (kt + 1) * 128)
                            ktb = kts[h][:, ksl].rearrange(
                                "p (j m) -> p j m", j=1
                            ).broadcast_to([128, 2, 128])
                            ktlob = ktlos[h][:, ksl].rearrange(
                                "p (j m) -> p j m", j=1
                            ).broadcast_to([128, 2, 128])
                            nc.tensor.matmul(
                                sps[:, s, c0:512],
                                lhsT=ktb,
                                rhs=qts[h][:, :, csl],
                                start=True,
                                stop=False,
                                perf_mode=PM.DoubleRow,
                            )
                            d0 = 128 * kt - 512 * jb  # diag block col offset
                            if 0 <= d0 < 512:
                                nc.tensor.matmul(
                                    sps[:, s, d0 : d0 + 128],
                                    lhsT=mlow,
                                    rhs=idsp,
                                    start=False,
                                    stop=False,
                                    perf_mode=PM.DoubleRow,
                                )
                                if d0 > c0:
                                    # fully-masked strip left of the diagonal
                                    nc.tensor.matmul(
                                        sps[:, s, c0:d0],
                                        lhsT=mful,
                                        rhs=idsp[:, :, 0 : d0 - c0],
                                        start=False,
                                        stop=False,
                                        perf_mode=PM.DoubleRow,
                                    )
                            nc.tensor.matmul(
                                sps[:, s, c0:512],
                                lhsT=ktlob,
                                rhs=qts[h][:, :, csl],
                                start=False,
                                stop=True,
                                perf_mode=PM.DoubleRow,
                            )
                        p_sb = phb.tile([128, 2, 512], BF16, tag="p", name="p_sb")
                        ex = nc.scalar.activation(
                            p_sb[:, :, c0:512],
                            sps[:, :, c0:512],
                            AF.Exp,
                            scale=c_inv8,
                            bias=c_bias,
                        )
                        if act_groups["L"]:
                            add_dep_helper(ex.ins, act_groups["L"][-1].ins, sync=False,
                                           reason="phase-B exps after phase-A Lns (ACT table)")
                            act_groups["L"] = []
                        p_sbs.append(p_sb)
                    if h == 0 and proj_pending:
                        emit_proj_fns[proj_pending.pop()]()
                    if pending is not None:
                        emit_y(*pending)
                    pending = (h, p_sbs, y_ps)
                emit_y(*pending)
                # transpose 4 t-tiles + c_proj + stage + dma out (deferred:
                # emitted after the next jb's first head S/exp so PE's
                # in-order queue doesn't stall ACT at jb boundaries)
                def emit_proj(jb=jb, last=False):
                  for q in range(4):
                    tt = 4 * jb + q
                    tsl = slice(tt * 128, (tt + 1) * 128)
                    pc = psc.tile([128, 2, 128], BF16, tag="pc", name="pc")
                    for cc in range(2):
                        nc.tensor.matmul(
                            pc[:, cc, :],
                            lhsT=y_t[:, tt, 2 * cc : 2 * cc + 2, :],
                            rhs=i128,
                            start=True,
                            stop=True,
                            is_transpose=True,
                        )
                    nc.vector.tensor_copy(out=yT[:, :, tsl], in_=pc)
                  for q in range(4):
                    tt = 4 * jb + q
                    tsl = slice(tt * 128, (tt + 1) * 128)
                    ost = ostage.tile([128, 1024], F32, tag="o", name="ost")
                    for eh in range(2):
                        po = psc.tile([128, 512], F32, tag="pc", name="po")
                        for cc in range(2):
                            nc.tensor.matmul(
                                po,
                                lhsT=yT[:, cc, tsl],
                                rhs=w2[:, cc, eh * 512 : (eh + 1) * 512],
                                start=(cc == 0),
                                stop=(cc == 1),
                            )
                        if last and (2 * q + eh) % 2 == 1:
                            nc.scalar.copy(ost[:, eh * 512 : (eh + 1) * 512], po)
                        else:
                            nc.vector.tensor_copy(
                                out=ost[:, eh * 512 : (eh + 1) * 512], in_=po
                            )
                        if last:
                            deng = nc.sync if (2 * q + eh) % 2 == 0 else nc.gpsimd
                            deng.dma_start(
                                out_d[tt][:, eh * 512 : (eh + 1) * 512],
                                ost[:, eh * 512 : (eh + 1) * 512],
                            )
                    if not last:
                        deng = nc.sync if q % 2 == 0 else nc.gpsimd
                        deng.dma_start(out_d[tt], ost)
                emit_proj_fns[jb] = emit_proj
                proj_pending.append(jb)
            while proj_pending:
                emit_proj_fns[proj_pending.pop()](last=True)
            rn_ctx.__exit__(None, None, None)
            ost_ctx.__exit__(None, None, None)
            phb_ctx.__exit__(None, None, None)
            abp_ctx.__exit__(None, None, None)
            a5_ctx.__exit__(None, None, None)
            mupool_ctx.__exit__(None, None, None)

    nc.compile()
    return nc


def make_inputs(x, w_attn, w_proj, delta):
    """Host-side prep: per-core input dicts (core = b*4 + g)."""
    x = np.asarray(x, dtype=np.float32)
    w_attn = np.asarray(w_attn, dtype=np.float32)
    w_proj = np.asarray(w_proj, dtype=np.float32)
    delta = np.asarray(delta, dtype=np.float32)

    inv_freq = 1.0 / (BASE ** (np.arange(D, dtype=np.float32) / D))
    t = np.arange(T, dtype=np.float64)
    freqs = (t[:, None] * inv_freq[None, :].astype(np.float64)).astype(np.float32)
    cosT = np.cos(freqs).T.astype(np.float32)  # (D, T)
    sinT = np.sin(freqs).T.astype(np.float32)
    trig = np.concatenate([cosT, sinT], axis=0).astype(ml_dtypes.bfloat16)
    d = np.clip(delta, -2.0 * math.pi, 0.0)

    # fp8 split of x per batch: [4cc, 128p, 2j, T] with c = cc*256+j*128+p
    def to_dr(mat):  # mat (T, C) -> (4, 128, 2, T)
        m = mat.T.reshape(4, 2, 128, T)  # (cc, j, p, t)
        return np.ascontiguousarray(m.transpose(0, 2, 1, 3))

    xw = [None] * B
    for b in range(B):
        xb = x[b]  # (T, C)
        x_hi = xb.astype(E4)
        x_lo = (xb - x_hi.astype(np.float32)).astype(E4)
        xw[b] = (to_dr(x_hi.astype(np.float32)).astype(E4),
                 to_dr(x_lo.astype(np.float32)).astype(E4))

    qw = w_attn[:C].reshape(H, D, C)
    kw = w_attn[C : 2 * C].reshape(H, D, C)
    vw = w_attn[2 * C :].reshape(H, D, C)

    # mask constants
    pp, jj, tk = np.meshgrid(
        np.arange(64), np.arange(2), np.arange(128), indexing="ij"
    )
    f = jj * 64 + pp
    mlow = np.where(tk > f, -240.0, 0.0).astype(E4)
    mful = np.full((64, 2, 128), -240.0, dtype=np.float32).astype(E4)
    idsp = (tk == f).astype(np.float32).astype(E4)
    i128 = np.eye(128, dtype=np.float32).astype(ml_dtypes.bfloat16)

    in_maps = []
    for core in range(N_CORES):
        b, g = divmod(core, HPC)
        heads = list(range(HPC * g, HPC * g + HPC))

        # wqk: (4cc, 128p, 2j, 512): col = h*128 + r; r<64 q_d else k_d
        wqk_full = np.empty((C, 512), dtype=np.float32)  # (c, col)
        for hi_, hg in enumerate(heads):
            wqk_full[:, hi_ * 128 : hi_ * 128 + 64] = qw[hg].T * WSCALE
            wqk_full[:, hi_ * 128 + 64 : hi_ * 128 + 128] = kw[hg].T * WSCALE
        wqk8 = wqk_full.astype(E4)
        wqk_dr = np.ascontiguousarray(
            wqk8.reshape(4, 2, 128, 512).transpose(0, 2, 1, 3)
        )

        wv_full = (
            vw[HPC * g : HPC * g + HPC].reshape(256, C).T * WSCALE
        )  # (c, 256)
        wv_hi = wv_full.astype(E4)
        wv_lo = (wv_full - wv_hi.astype(np.float32)).astype(E4)
        wvh_dr = np.ascontiguousarray(
            wv_hi.reshape(4, 2, 128, 256).transpose(0, 2, 1, 3)
        )
        wvl_dr = np.ascontiguousarray(
            wv_lo.reshape(4, 2, 128, 256).transpose(0, 2, 1, 3)
        )

        ab = np.stack(
            [
                np.concatenate(
                    [
                        np.cos(freqs + d[hg][None, :]).T,
                        np.sin(freqs + d[hg][None, :]).T,
                    ],
                    axis=0,
                ).astype(ml_dtypes.bfloat16)
                for hg in heads
            ],
            axis=0,
        )  # (4, 128, T)

        # w2: (2cc, 128p, 1024e): channel c_local = cc*128 + p of this group's
        # 256 y channels; y channel (h_local, dd) flattened h_local*64+dd
        w2g = w_proj[:, 256 * g : 256 * (g + 1)]  # (e, 256)
        w2_dr = np.ascontiguousarray((w2g.T / WSCALE).reshape(2, 128, 1024)).astype(ml_dtypes.bfloat16)

        in_maps.append(
            {
                "xhi": xw[b][0],
                "xlo": xw[b][1],
                "wqk": wqk_dr,
                "wvh": wvh_dr,
                "wvl": wvl_dr,
                "trig": trig,
                "ab": ab,
                "mlow": mlow,
                "mful": mful,
                "idsp": idsp,
                "i128": i128,
                "w2": w2_dr,
            }
        )
    return in_maps


_NC_CACHE = []


def _get_nc():
    if not _NC_CACHE:
        _NC_CACHE.append(build_module())
    return _NC_CACHE[0]


def kernel(x, w_attn, w_proj, delta, _trace=False):
    in_maps = make_inputs(x, w_attn, w_proj, delta)
    nc = _get_nc()
    res = None
    outs = None
    last_err = None
    for attempt in range(3):
        try:
            res = bass_utils.run_bass_kernel_spmd(
                nc, in_maps, core_ids=list(range(N_CORES)), trace=_trace
            )
            outs = [np.asarray(r["out"]).reshape(T, C) for r in res.results]
            break
        except Exception as e:
            last_err = e
            if "unrecoverable" not in str(e).lower() or attempt == 2:
                raise
            import time as _time

            _time.sleep(2.0)
    assert outs is not None, last_err
    if _trace:
        kernel.last_results = res
    full = np.zeros((B, T, C), dtype=np.float32)
    for core in range(N_CORES):
        full[core // HPC] += outs[core]
    return full


# revision 6
# speedup vs baseline: 1.0546x; 1.0045x over previous
"""Trainium2 Bass kernel for causal self-attention with PoPE (v2, fp8).

Reference (B=2, T=2048, C=1024, H=16, D=64):
  qkv = x @ w_attn.T ; mu = softplus(q|k)
  q_aug = mu_q * [cos(tw), sin(tw)] ; k_aug = mu_k * [cos(tw+d), sin(tw+d)]
  att = softmax_causal((q_aug . k_aug)/8) ; out = (att @ v) @ w_proj.T

Sharding: 8 cores = 2 batches x 4 head-groups (4 heads each); host sums
the 4 c_proj partials per batch.

Design (fp8 DoubleRow matmuls at 0.5 cyc/col with K=256/pass; ACT
exp is the bottleneck engine at ~0.833 ns/col):
  - qk proj: fp8e4m3 DR, x_hi and 64*w quantized host-side
  - v  proj: fp8 DR 3-term (x_hi@wv_hi + x_hi@wv_lo + x_lo@wv_hi)
  - softplus: Exp(scale=1/64) psum->mu bf16, then Ln(bias=1) in place;
    all Exps before all Lns so the ACT table loads only 3x total
  - Qt: hi+lo fp8 residual pair in the DR j-dim; Kt: hi and lo each
    replicated across j via stride-0 APs => S = Qt.(Kt_hi+Kt_lo) exact
    to fp8-pair precision in 2 DR matmuls per 128-key tile
  - causal mask folded into the S psum as -240-additive fp8 matmuls
    (lower-strict const x split-identity), so exp gives 0; no mask ops
  - P = exp(S/8 - 2.5) -> bf16; y = P^T @ V_aug bf16 per 128-tq tile
    => y [tq, 64ch|denom] in psum; per-partition normalize (DVE recip +
    broadcast mul); PE transpose-mode -> yT; c_proj in bf16
  - tq blocks processed high-to-low so the large jb=3 exp workload
    covers Qt/Kt formation; heads software-pipelined (S/exp of head h+1
    emitted before y of head h); c_proj of block jb deferred into block
    jb-1's first head (PE queues are in-order)
  - gpsimd never touches PSUM (ISA restriction); residuals via
    tensor_sub; fp8/bf16 conversions split across DVE/Pool
Engine busy (CoreSim): ACT ~97us (bottleneck), PE ~83us, span ~130us.
End-to-end rel err ~1.07e-2 (budget 2e-2): dominant terms are single-
level fp8 x/w in the qk projection (~1.0%) and fp8 Kt (~0.3% after the
two-sided split).
"""

import math
import os
import sys

import numpy as np
import ml_dtypes

for _p in ("/opt/trn_rl_repo",):
    if _p not in sys.path and os.path.isdir(_p):
        sys.path.insert(0, _p)

import concourse.tile as tile
from concourse import bacc
from concourse import mybir
from concourse import bass_utils

B, T, C = 2, 2048, 1024
H, D = 16, 64
BASE = 10000.0
N_CORES = 8
HPC = 4  # heads per core
NTT = 16  # t tiles of 128

F32 = mybir.dt.float32
F32R = mybir.dt.float32r
BF16 = mybir.dt.bfloat16
FP8 = mybir.dt.float8e4
AF = mybir.ActivationFunctionType
PM = mybir.MatmulPerfMode
OP = mybir.AluOpType
E4 = ml_dtypes.float8_e4m3
WSCALE = 64.0


def build_module():
    nc = bacc.Bacc(
        "TRN2", target_bir_lowering=False, debug=False, num_devices=N_CORES
    )

    xhi_d = nc.dram_tensor("xhi", (4, 128, 2, T), FP8, kind="ExternalInput").ap()
    xlo_d = nc.dram_tensor("xlo", (4, 128, 2, T), FP8, kind="ExternalInput").ap()
    wqk_d = nc.dram_tensor("wqk", (4, 128, 2, 512), FP8, kind="ExternalInput").ap()
    wvh_d = nc.dram_tensor("wvh", (4, 128, 2, 256), FP8, kind="ExternalInput").ap()
    wvl_d = nc.dram_tensor("wvl", (4, 128, 2, 256), FP8, kind="ExternalInput").ap()
    trig_d = nc.dram_tensor("trig", (128, T), BF16, kind="ExternalInput").ap()
    ab_d = nc.dram_tensor("ab", (HPC, 128, T), BF16, kind="ExternalInput").ap()
    mlow_d = nc.dram_tensor("mlow", (64, 2, 128), FP8, kind="ExternalInput").ap()
    mful_d = nc.dram_tensor("mful", (64, 2, 128), FP8, kind="ExternalInput").ap()
    idsp_d = nc.dram_tensor("idsp", (64, 2, 128), FP8, kind="ExternalInput").ap()
    i128_d = nc.dram_tensor("i128", (128, 128), BF16, kind="ExternalInput").ap()
    w2_d = nc.dram_tensor("w2", (2, 128, 1024), BF16, kind="ExternalInput").ap()
    out_d = nc.dram_tensor("out", (NTT, 128, 1024), F32, kind="ExternalOutput").ap()

    with tile.TileContext(nc) as tc:
        with (
            tc.tile_pool(name="persist", bufs=1) as persist,
            tc.tile_pool(name="qkpool", bufs=1) as qkpool,
            tc.tile_pool(name="ps2", bufs=2, space="PSUM") as ps2,
            tc.tile_pool(name="psy", bufs=2, space="PSUM") as psy,
            tc.tile_pool(name="psc", bufs=2, space="PSUM") as psc,
        ):
            # ---- persistent constants / outputs-in-sbuf ----
            mlow = persist.tile([64, 2, 128], FP8)
            mful = persist.tile([64, 2, 128], FP8)
            idsp = persist.tile([64, 2, 128], FP8)
            i128 = persist.tile([128, 128], BF16)
            trig = persist.tile([128, T], BF16)
            c_inv64 = persist.tile([128, 1], F32)
            c_inv8 = persist.tile([128, 1], F32)
            c_bias = persist.tile([128, 1], F32)
            nc.gpsimd.memset(c_inv64, 1.0 / WSCALE)
            nc.gpsimd.memset(c_inv8, 0.125)
            nc.gpsimd.memset(c_bias, -2.5)
            v_aug = persist.tile([128, NTT, HPC, 65], BF16)
            nc.vector.memset(
                v_aug.rearrange("p a b c -> p (a b) c")[:, :, 64:65], 1.0
            )
            # per-head S inputs (fp8): qt = [hi|lo] in DR j-dim; kt single
            qts = [qkpool.tile([128, 2, T], FP8, name=f"qt{h}") for h in range(HPC)]
            kts = [qkpool.tile([128, T], FP8, name=f"kt{h}") for h in range(HPC)]
            ktlos = [qkpool.tile([128, T], FP8, name=f"ktlo{h}") for h in range(HPC)]
            y_t = persist.tile([128, NTT, HPC, 64], BF16)
            yT = persist.tile([128, 2, T], BF16)
            w2 = persist.tile([128, 2, 1024], BF16)
            
            from concourse.tile_rust import add_dep_helper
            act_groups = {"E": [], "L": []}
            # ---------------- Phase A: QKV projection ----------------
            mupool_ctx = tc.tile_pool(name="mupool", bufs=1)
            mupool = mupool_ctx.__enter__()
            mu = [mupool.tile([128, T], BF16, name=f"mu{h}") for h in range(HPC)]

            a5_ctx = tc.tile_pool(name="a5", bufs=2)
            a5 = a5_ctx.__enter__()
            abp_ctx = tc.tile_pool(name="abp", bufs=2)
            abp = abp_ctx.__enter__()
            phb_ctx = tc.tile_pool(name="phB", bufs=17)
            phb = phb_ctx.__enter__()
            ost_ctx = tc.tile_pool(name="ostage", bufs=3)
            ostage = ost_ctx.__enter__()
            rn_ctx = tc.tile_pool(name="rn", bufs=2)
            rnp = rn_ctx.__enter__()
            pha_ctx = tc.tile_pool(name="phA", bufs=1)
            pha = pha_ctx.__enter__()
            xhi = pha.tile([128, 4, 2, T], FP8)
            xlo = pha.tile([128, 4, 2, T], FP8)
            wqk = pha.tile([128, 4, 2, 512], FP8)
            wvh = pha.tile([128, 4, 2, 256], FP8)
            wvl = pha.tile([128, 4, 2, 256], FP8)
            nc.sync.dma_start(wqk, wqk_d.rearrange("o p j e -> p o j e"))
            engs = [nc.sync, nc.gpsimd, nc.sync, nc.gpsimd]
            for th in range(2):
                tsl = slice(th * 1024, th * 1024 + 1024)
                for cc in range(4):
                    engs[cc].dma_start(xhi[:, cc, :, tsl], xhi_d[cc][:, :, tsl])
            nc.sync.dma_start(wvh, wvh_d.rearrange("o p j e -> p o j e"))
            nc.sync.dma_start(wvl, wvl_d.rearrange("o p j e -> p o j e"))
            for th in range(2):
                tsl = slice(th * 1024, th * 1024 + 1024)
                for cc in range(4):
                    engs[(cc + 1) % 4].dma_start(
                        xlo[:, cc, :, tsl], xlo_d[cc][:, :, tsl]
                    )
            nc.gpsimd.dma_start(trig, trig_d)
            nc.gpsimd.dma_start(mlow, mlow_d)
            nc.gpsimd.dma_start(mful, mful_d)
            nc.gpsimd.dma_start(idsp, idsp_d)
            nc.gpsimd.dma_start(i128, i128_d)
            nc.gpsimd.dma_start(w2, w2_d.rearrange("o p e -> p o e"))

            # qk: per (head, 1024-block): psum [128,1024], 4 DR matmuls over cc
            # v: out[t, 256] per t-tile, 4 tiles per psum; 3-term fp8 DR.
            # Interleave qk and v allocations so PE has v work while ACT
            # drains softplus.
            def emit_qk(h, tb):
                ts_ = slice(tb * 1024, tb * 1024 + 1024)
                qk_ps = ps2.tile([128, 1024], F32, tag="s2", name="qk_ps")
                for half in range(2):
                    hs = slice(tb * 1024 + half * 512, tb * 1024 + half * 512 + 512)
                    for cc in range(4):
                        nc.tensor.matmul(
                            qk_ps[:, half * 512 : half * 512 + 512],
                            lhsT=wqk[:, cc, :, h * 128 : (h + 1) * 128],
                            rhs=xhi[:, cc, :, hs],
                            start=(cc == 0),
                            stop=(cc == 3),
                            perf_mode=PM.DoubleRow,
                        )
                # softplus = ln(exp(q/64)+1): exp -> mu (bf16), ln batched
                # later so the ACT table doesn't ping-pong between sets
                act_groups["E"].append(
                    nc.scalar.activation(mu[h][:, ts_], qk_ps, AF.Exp, scale=c_inv64)
                )

            def emit_v(tq):  # tq = t-pair index 0..7
                v_ps = psc.tile([128, 2, 256], F32, tag="pc", name="v_ps")
                for s in range(2):
                    tt = 2 * tq + s
                    tsl = slice(tt * 128, (tt + 1) * 128)
                    terms = ((xhi, wvh), (xhi, wvl), (xlo, wvh))
                    n = 0
                    for cc in range(4):
                        for (xx, ww) in terms:
                            nc.tensor.matmul(
                                v_ps[:, s, :],
                                lhsT=xx[:, cc, :, tsl],
                                rhs=ww[:, cc],
                                start=(n == 0),
                                stop=(n == 11),
                                perf_mode=PM.DoubleRow,
                            )
                            n += 1
                nc.vector.tensor_copy(
                    out=v_aug[:, 2 * tq : 2 * tq + 2, :, 0:64],
                    in_=v_ps.rearrange("p s (h e) -> p s h e", h=HPC),
                )

            for h in range(HPC):
                emit_qk(h, 0)
                emit_qk(h, 1)
            for tq in range(8):
                emit_v(tq)
            for h in range(HPC):
                ln = nc.scalar.activation(mu[h], mu[h], AF.Ln, bias=1.0)
                add_dep_helper(ln.ins, act_groups["E"][-1].ins, sync=False,
                               reason="group phase-A Lns after Exps (ACT table)")
                act_groups["L"].append(ln)

            pha_ctx.__exit__(None, None, None)

            # ------- Phase A.5: Qt/Kt formation (per head) -------
            for h in range(HPC):
                abh = abp.tile([128, T], BF16, tag="ab", name="abh")
                nc.sync.dma_start(abh, ab_d[h])
                # musw = [mu_k ; mu_q] (swapped halves) via DVE shuffles;
                # processed in 1024-col halves, high half first (jb runs
                # descending, so high-t qt cols are needed first; kt pair 0
                # needs low-t keys first -> kt low half first)
                musw = a5.tile([128, T], BF16, tag="musw", name="musw")
                qtf = a5.tile([128, T], BF16, tag="qtf", name="qtf")
                ktf = a5.tile([128, T], BF16, tag="ktf", name="ktf")
                idm = list(range(32))
                qeng = nc.vector if h == 0 else nc.gpsimd
                for hb in (0,) if h == 0 else ((1, 0)):
                    # kt chain on Pool (low keys first: S pair 0 reads them)
                    ts_ = slice(hb * 1024, hb * 1024 + 1024)
                    if h == 0:
                        nc.vector.stream_shuffle(
                            musw[0:64, ts_], mu[h][64:128, ts_], idm
                        )
                    else:
                        nc.sync.dma_start(musw[0:64, ts_], mu[h][64:128, ts_])
                    nc.gpsimd.tensor_mul(
                        ktf[0:64, ts_], musw[0:64, ts_], abh[0:64, ts_]
                    )
                    nc.gpsimd.tensor_mul(
                        ktf[64:128, ts_], mu[h][64:128, ts_], abh[64:128, ts_]
                    )
                    nc.gpsimd.tensor_copy(out=kts[h][:, ts_], in_=ktf[:, ts_])
                    nc.gpsimd.tensor_sub(
                        ktlos[h][:, ts_], ktf[:, ts_], kts[h][:, ts_]
                    )
                for hb in ((1, 0) if h != 0 else (1,)):
                    # qt chain on DVE (h0: incl. fp8 ops, parallel with Pool)
                    ts_ = slice(hb * 1024, hb * 1024 + 1024)
                    if h == 0:
                        nc.vector.stream_shuffle(
                            musw[64:128, ts_], mu[h][0:64, ts_], idm
                        )
                    else:
                        nc.sync.dma_start(musw[64:128, ts_], mu[h][0:64, ts_])
                    nc.vector.tensor_mul(
                        qtf[0:64, ts_], mu[h][0:64, ts_], trig[0:64, ts_]
                    )
                    nc.vector.tensor_mul(
                        qtf[64:128, ts_], musw[64:128, ts_], trig[64:128, ts_]
                    )
                    qeng.tensor_copy(
                        out=qts[h][:, 0, ts_], in_=qtf[:, ts_]
                    )
                    qeng.tensor_sub(
                        qts[h][:, 1, ts_], qtf[:, ts_], qts[h][:, 0, ts_]
                    )
                if h == 0:
                    for ts_ in (slice(1024, 2048), slice(0, 1024)):
                        nc.vector.stream_shuffle(
                            musw[0:64, ts_] if ts_.start == 1024 else musw[64:128, ts_],
                            mu[h][64:128, ts_] if ts_.start == 1024 else mu[h][0:64, ts_],
                            idm,
                        )
                    ts_ = slice(1024, 2048)
                    nc.gpsimd.tensor_mul(
                        ktf[0:64, ts_], musw[0:64, ts_], abh[0:64, ts_]
                    )
                    nc.gpsimd.tensor_mul(
                        ktf[64:128, ts_], mu[h][64:128, ts_], abh[64:128, ts_]
                    )
                    nc.gpsimd.tensor_copy(out=kts[h][:, ts_], in_=ktf[:, ts_])
                    nc.gpsimd.tensor_sub(
                        ktlos[h][:, ts_], ktf[:, ts_], kts[h][:, ts_]
                    )
                    ts_ = slice(0, 1024)
                    nc.vector.tensor_mul(
                        qtf[0:64, ts_], mu[h][0:64, ts_], trig[0:64, ts_]
                    )
                    nc.vector.tensor_mul(
                        qtf[64:128, ts_], musw[64:128, ts_], trig[64:128, ts_]
                    )
                    nc.vector.tensor_copy(out=qts[h][:, 0, ts_], in_=qtf[:, ts_])
                    nc.vector.tensor_sub(
                        qts[h][:, 1, ts_], qtf[:, ts_], qts[h][:, 0, ts_]
                    )

            # ------- Phase B: attention + transpose + c_proj, per tq block ----
            proj_pending = []
            emit_proj_fns = {}
            pending = None
            for jb in (3, 2, 1, 0):
                cols = slice(jb * 512, (jb + 1) * 512)
                def emit_y(h, p_sbs, y_ps, yjb=None):
                    yjb = jb if yjb is None else yjb
                    for q in range(4):
                        jt = 4 * yjb + q
                        for kt in range(jt + 1):
                            nc.tensor.matmul(
                                y_ps[:, q, :],
                                lhsT=p_sbs[kt // 2][:, kt % 2, q * 128 : (q + 1) * 128],
                                rhs=v_aug[:, kt, h, :],
                                start=(kt == 0),
                                stop=(kt == jt),
                            )
                    rn = rnp.tile([128, 4], F32, tag="rn", name="rn")
                    nc.vector.reciprocal(
                        rn, y_ps[:, :, 64:65].rearrange("p a b -> p (a b)")
                    )
                    rnb = rn.rearrange("p (a b) -> p a b", b=1).broadcast_to(
                        [128, 4, 64]
                    )
                    nc.vector.tensor_mul(
                        y_t[:, 4 * yjb : 4 * yjb + 4, h, :], y_ps[:, :, 0:64], rnb
                    )

                for h in range(HPC):
                    y_ps = psy.tile([128, 4, 65], F32, tag="y1", name="y_ps")
                    p_sbs = []
                    for p in range(2 * jb + 2):
                        c0 = max(0, 256 * p - 512 * jb)
                        csl = slice(jb * 512 + c0, (jb + 1) * 512)
                        sps = ps2.tile([128, 2, 512], F32, tag="s2", name="sps")
                        for s in range(2):
                            kt = 2 * p + s
                            ksl = slice(kt * 128, (kt + 1) * 128)
                            ktb = kts[h][:, ksl].rearrange(
                                "p (j m) -> p j m", j=1
                            ).broadcast_to([128, 2, 128])
                            ktlob = ktlos[h][:, ksl].rearrange(
                                "p (j m) -> p j m", j=1
                            ).broadcast_to([128, 2, 128])
                            nc.tensor.matmul(
                                sps[:, s, c0:512],
                                lhsT=ktb,
                                rhs=qts[h][:, :, csl],
                                start=True,
                                stop=False,
                                perf_mode=PM.DoubleRow,
                            )
                            d0 = 128 * kt - 512 * jb  # diag block col offset
                            if 0 <= d0 < 512:
                                nc.tensor.matmul(
                                    sps[:, s, d0 : d0 + 128],
                                    lhsT=mlow,
                                    rhs=idsp,
                                    start=False,
                                    stop=False,
                                    perf_mode=PM.DoubleRow,
                                )
                                if d0 > c0:
                                    # fully-masked strip left of the diagonal
                                    nc.tensor.matmul(
                                        sps[:, s, c0:d0],
                                        lhsT=mful,
                                        rhs=idsp[:, :, 0 : d0 - c0],
                                        start=False,
                                        stop=False,
                                        perf_mode=PM.DoubleRow,
                                    )
                            nc.tensor.matmul(
                                sps[:, s, c0:512],
                                lhsT=ktlob,
                                rhs=qts[h][:, :, csl],
                                start=False,
                                stop=True,
                                perf_mode=PM.DoubleRow,
                            )
                        p_sb = phb.tile([128, 2, 512], BF16, tag="p", name="p_sb")
                        ex = nc.scalar.activation(
                            p_sb[:, :, c0:512],
                            sps[:, :, c0:512],
                            AF.Exp,
                            scale=c_inv8,
                            bias=c_bias,
                        )
                        if act_groups["L"]:
                            add_dep_helper(ex.ins, act_groups["L"][-1].ins, sync=False,
                                           reason="phase-B exps after phase-A Lns (ACT table)")
                            act_groups["L"] = []
                        p_sbs.append(p_sb)
                    if pending is not None:
                        emit_y(*pending)
                        pending = None
                    if h == 0 and proj_pending:
                        emit_proj_fns[proj_pending.pop()]()
                    pending = (h, p_sbs, y_ps, jb)
                # transpose 4 t-tiles + c_proj + stage + dma out (deferred:
                # emitted after the next jb's first head S/exp so PE's
                # in-order queue doesn't stall ACT at jb boundaries)
                def emit_proj(jb=jb, last=False):
                  for q in range(4):
                    tt = 4 * jb + q
                    tsl = slice(tt * 128, (tt + 1) * 128)
                    pc = psc.tile([128, 2, 128], BF16, tag="pc", name="pc")
                    for cc in range(2):
                        nc.tensor.matmul(
                            pc[:, cc, :],
                            lhsT=y_t[:, tt, 2 * cc : 2 * cc + 2, :],
                            rhs=i128,
                            start=True,
                            stop=True,
                            is_transpose=True,
                        )
                    nc.vector.tensor_copy(out=yT[:, :, tsl], in_=pc)
                  for q in range(4):
                    tt = 4 * jb + q
                    tsl = slice(tt * 128, (tt + 1) * 128)
                    ost = ostage.tile([128, 1024], F32, tag="o", name="ost")
                    for eh in range(2):
                        po = psc.tile([128, 512], F32, tag="pc", name="po")
                        for cc in range(2):
                            nc.tensor.matmul(
                                po,
                                lhsT=yT[:, cc, tsl],
                                rhs=w2[:, cc, eh * 512 : (eh + 1) * 512],
                                start=(cc == 0),
                                stop=(cc == 1),
                            )
                        if last and (2 * q + eh) % 2 == 1:
                            nc.scalar.copy(ost[:, eh * 512 : (eh + 1) * 512], po)
                        else:
                            nc.vector.tensor_copy(
                                out=ost[:, eh * 512 : (eh + 1) * 512], in_=po
                            )
                        if last:
                            deng = nc.sync if (2 * q + eh) % 2 == 0 else nc.gpsimd
                            deng.dma_start(
                                out_d[tt][:, eh * 512 : (eh + 1) * 512],
                                ost[:, eh * 512 : (eh + 1) * 512],
                            )
                    if not last:
                        deng = nc.sync if q % 2 == 0 else nc.gpsimd
                        deng.dma_start(out_d[tt], ost)
                emit_proj_fns[jb] = emit_proj
                proj_pending.append(jb)
            if pending is not None:
                emit_y(*pending)
            while proj_pending:
                emit_proj_fns[proj_pending.pop()](last=True)
            rn_ctx.__exit__(None, None, None)
            ost_ctx.__exit__(None, None, None)
            phb_ctx.__exit__(None, None, None)
            abp_ctx.__exit__(None, None, None)
            a5_ctx.__exit__(None, None, None)
            mupool_ctx.__exit__(None, None, None)

    nc.compile()
    return nc


def make_inputs(x, w_attn, w_proj, delta):
    """Host-side prep: per-core input dicts (core = b*4 + g)."""
    x = np.asarray(x, dtype=np.float32)
    w_attn = np.asarray(w_attn, dtype=np.float32)
    w_proj = np.asarray(w_proj, dtype=np.float32)
    delta = np.asarray(delta, dtype=np.float32)

    inv_freq = 1.0 / (BASE ** (np.arange(D, dtype=np.float32) / D))
    t = np.arange(T, dtype=np.float64)
    freqs = (t[:, None] * inv_freq[None, :].astype(np.float64)).astype(np.float32)
    cosT = np.cos(freqs).T.astype(np.float32)  # (D, T)
    sinT = np.sin(freqs).T.astype(np.float32)
    trig = np.concatenate([cosT, sinT], axis=0).astype(ml_dtypes.bfloat16)
    d = np.clip(delta, -2.0 * math.pi, 0.0)

    # fp8 split of x per batch: [4cc, 128p, 2j, T] with c = cc*256+j*128+p
    def to_dr(mat):  # mat (T, C) -> (4, 128, 2, T)
        m = mat.T.reshape(4, 2, 128, T)  # (cc, j, p, t)
        return np.ascontiguousarray(m.transpose(0, 2, 1, 3))

    xw = [None] * B
    for b in range(B):
        xb = x[b]  # (T, C)
        x_hi = xb.astype(E4)
        x_lo = (xb - x_hi.astype(np.float32)).astype(E4)
        xw[b] = (to_dr(x_hi.astype(np.float32)).astype(E4),
                 to_dr(x_lo.astype(np.float32)).astype(E4))

    qw = w_attn[:C].reshape(H, D, C)
    kw = w_attn[C : 2 * C].reshape(H, D, C)
    vw = w_attn[2 * C :].reshape(H, D, C)

    # mask constants
    pp, jj, tk = np.meshgrid(
        np.arange(64), np.arange(2), np.arange(128), indexing="ij"
    )
    f = jj * 64 + pp
    mlow = np.where(tk > f, -240.0, 0.0).astype(E4)
    mful = np.full((64, 2, 128), -240.0, dtype=np.float32).astype(E4)
    idsp = (tk == f).astype(np.float32).astype(E4)
    i128 = np.eye(128, dtype=np.float32).astype(ml_dtypes.bfloat16)

    in_maps = []
    for core in range(N_CORES):
        b, g = divmod(core, HPC)
        heads = list(range(HPC * g, HPC * g + HPC))

        # wqk: (4cc, 128p, 2j, 512): col = h*128 + r; r<64 q_d else k_d
        wqk_full = np.empty((C, 512), dtype=np.float32)  # (c, col)
        for hi_, hg in enumerate(heads):
            wqk_full[:, hi_ * 128 : hi_ * 128 + 64] = qw[hg].T * WSCALE
            wqk_full[:, hi_ * 128 + 64 : hi_ * 128 + 128] = kw[hg].T * WSCALE
        wqk8 = wqk_full.astype(E4)
        wqk_dr = np.ascontiguousarray(
            wqk8.reshape(4, 2, 128, 512).transpose(0, 2, 1, 3)
        )

        wv_full = (
            vw[HPC * g : HPC * g + HPC].reshape(256, C).T * WSCALE
        )  # (c, 256)
        wv_hi = wv_full.astype(E4)
        wv_lo = (wv_full - wv_hi.astype(np.float32)).astype(E4)
        wvh_dr = np.ascontiguousarray(
            wv_hi.reshape(4, 2, 128, 256).transpose(0, 2, 1, 3)
        )
        wvl_dr = np.ascontiguousarray(
            wv_lo.reshape(4, 2, 128, 256).transpose(0, 2, 1, 3)
        )

        ab = np.stack(
            [
                np.concatenate(
                    [
                        np.cos(freqs + d[hg][None, :]).T,
                        np.sin(freqs + d[hg][None, :]).T,
                    ],
                    axis=0,
                ).astype(ml_dtypes.bfloat16)
                for hg in heads
            ],
            axis=0,
        )  # (4, 128, T)

        # w2: (2cc, 128p, 1024e): channel c_local = cc*128 + p of this group's
        # 256 y channels; y channel (h_local, dd) flattened h_local*64+dd
        w2g = w_proj[:, 256 * g : 256 * (g + 1)]  # (e, 256)
        w2_dr = np.ascontiguousarray((w2g.T / WSCALE).reshape(2, 128, 1024)).astype(ml_dtypes.bfloat16)

        in_maps.append(
            {
                "xhi": xw[b][0],
                "xlo": xw[b][1],
                "wqk": wqk_dr,
                "wvh": wvh_dr,
                "wvl": wvl_dr,
                "trig": trig,
                "ab": ab,
                "mlow": mlow,
                "mful": mful,
                "idsp": idsp,
                "i128": i128,
                "w2": w2_dr,
            }
        )
    return in_maps


_NC_CACHE = []


def _get_nc():
    if not _NC_CACHE:
        _NC_CACHE.append(build_module())
    return _NC_CACHE[0]


def kernel(x, w_attn, w_proj, delta, _trace=False):
    in_maps = make_inputs(x, w_attn, w_proj, delta)
    nc = _get_nc()
    res = None
    outs = None
    last_err = None
    for attempt in range(3):
        try:
            res = bass_utils.run_bass_kernel_spmd(
                nc, in_maps, core_ids=list(range(N_CORES)), trace=_trace
            )
            outs = [np.asarray(r["out"]).reshape(T, C) for r in res.results]
            break
        except Exception as e:
            last_err = e
            if "unrecoverable" not in str(e).lower() or attempt == 2:
                raise
            import time as _time

            _time.sleep(2.0)
    assert outs is not None, last_err
    if _trace:
        kernel.last_results = res
    full = np.zeros((B, T, C), dtype=np.float32)
    for core in range(N_CORES):
        full[core // HPC] += outs[core]
    return full


# revision 7
# speedup vs baseline: 1.0634x; 1.0083x over previous
"""Trainium2 Bass kernel for causal self-attention with PoPE (v2, fp8).

Reference (B=2, T=2048, C=1024, H=16, D=64):
  qkv = x @ w_attn.T ; mu = softplus(q|k)
  q_aug = mu_q * [cos(tw), sin(tw)] ; k_aug = mu_k * [cos(tw+d), sin(tw+d)]
  att = softmax_causal((q_aug . k_aug)/8) ; out = (att @ v) @ w_proj.T

Sharding: 8 cores = 2 batches x 4 head-groups (4 heads each); host sums
the 4 c_proj partials per batch.

Design (fp8 DoubleRow matmuls at 0.5 cyc/col with K=256/pass; ACT
exp is the bottleneck engine at ~0.833 ns/col):
  - qk proj: fp8e4m3 DR, x_hi and 64*w quantized host-side
  - v  proj: fp8 DR 3-term (x_hi@wv_hi + x_hi@wv_lo + x_lo@wv_hi)
  - softplus: Exp(scale=1/64) psum->mu bf16, then Ln(bias=1) in place;
    all Exps before all Lns so the ACT table loads only 3x total
  - Qt: hi+lo fp8 residual pair in the DR j-dim; Kt: hi and lo each
    replicated across j via stride-0 APs => S = Qt.(Kt_hi+Kt_lo) exact
    to fp8-pair precision in 2 DR matmuls per 128-key tile
  - causal mask folded into the S psum as -240-additive fp8 matmuls
    (lower-strict const x split-identity), so exp gives 0; no mask ops
  - P = exp(S/8 - 2.5) -> bf16; y = P^T @ V_aug bf16 per 128-tq tile
    => y [tq, 64ch|denom] in psum; per-partition normalize (DVE recip +
    broadcast mul); PE transpose-mode -> yT; c_proj in bf16
  - tq blocks processed high-to-low so the large jb=3 exp workload
    covers Qt/Kt formation; heads software-pipelined (S/exp of head h+1
    emitted before y of head h); c_proj of block jb deferred into block
    jb-1's first head (PE queues are in-order)
  - gpsimd never touches PSUM (ISA restriction); residuals via
    tensor_sub; fp8/bf16 conversions split across DVE/Pool
Engine busy (CoreSim): ACT ~97us (bottleneck), PE ~83us, span ~130us.
End-to-end rel err ~1.07e-2 (budget 2e-2): dominant terms are single-
level fp8 x/w in the qk projection (~1.0%) and fp8 Kt (~0.3% after the
two-sided split).
"""

import math
import os
import sys

import numpy as np
import ml_dtypes

for _p in ("/opt/trn_rl_repo",):
    if _p not in sys.path and os.path.isdir(_p):
        sys.path.insert(0, _p)

import concourse.tile as tile
from concourse import bacc
from concourse import mybir
from concourse import bass_utils

B, T, C = 2, 2048, 1024
H, D = 16, 64
BASE = 10000.0
N_CORES = 8
HPC = 4  # heads per core
NTT = 16  # t tiles of 128

F32 = mybir.dt.float32
F32R = mybir.dt.float32r
BF16 = mybir.dt.bfloat16
FP8 = mybir.dt.float8e4
AF = mybir.ActivationFunctionType
PM = mybir.MatmulPerfMode
OP = mybir.AluOpType
E4 = ml_dtypes.float8_e4m3
WSCALE = 64.0


def build_module():
    nc = bacc.Bacc(
        "TRN2", target_bir_lowering=False, debug=False, num_devices=N_CORES
    )

    xhi_d = nc.dram_tensor("xhi", (4, 128, 2, T), FP8, kind="ExternalInput").ap()
    xlo_d = nc.dram_tensor("xlo", (4, 128, 2, T), FP8, kind="ExternalInput").ap()
    wqk_d = nc.dram_tensor("wqk", (4, 128, 2, 512), FP8, kind="ExternalInput").ap()
    wvh_d = nc.dram_tensor("wvh", (4, 128, 2, 256), FP8, kind="ExternalInput").ap()
    wvl_d = nc.dram_tensor("wvl", (4, 128, 2, 256), FP8, kind="ExternalInput").ap()
    trig_d = nc.dram_tensor("trig", (128, T), BF16, kind="ExternalInput").ap()
    ab_d = nc.dram_tensor("ab", (HPC, 128, T), BF16, kind="ExternalInput").ap()
    mlow_d = nc.dram_tensor("mlow", (64, 2, 128), FP8, kind="ExternalInput").ap()
    mful_d = nc.dram_tensor("mful", (64, 2, 128), FP8, kind="ExternalInput").ap()
    idsp_d = nc.dram_tensor("idsp", (64, 2, 128), FP8, kind="ExternalInput").ap()
    i128_d = nc.dram_tensor("i128", (128, 128), BF16, kind="ExternalInput").ap()
    w2_d = nc.dram_tensor("w2", (2, 128, 1024), BF16, kind="ExternalInput").ap()
    out_d = nc.dram_tensor("out", (NTT, 128, 1024), F32, kind="ExternalOutput").ap()

    with tile.TileContext(nc) as tc:
        with (
            tc.tile_pool(name="persist", bufs=1) as persist,
            tc.tile_pool(name="qkpool", bufs=1) as qkpool,
            tc.tile_pool(name="ps2", bufs=2, space="PSUM") as ps2,
            tc.tile_pool(name="psy", bufs=2, space="PSUM") as psy,
            tc.tile_pool(name="psc", bufs=2, space="PSUM") as psc,
        ):
            # ---- persistent constants / outputs-in-sbuf ----
            mlow = persist.tile([64, 2, 128], FP8)
            mful = persist.tile([64, 2, 128], FP8)
            idsp = persist.tile([64, 2, 128], FP8)
            i128 = persist.tile([128, 128], BF16)
            trig = persist.tile([128, T], BF16)
            c_inv64 = persist.tile([128, 1], F32)
            c_inv8 = persist.tile([128, 1], F32)
            c_bias = persist.tile([128, 1], F32)
            nc.gpsimd.memset(c_inv64, 1.0 / WSCALE)
            nc.gpsimd.memset(c_inv8, 0.125)
            nc.gpsimd.memset(c_bias, -2.5)
            v_aug = persist.tile([128, NTT, HPC, 65], BF16)
            nc.vector.memset(
                v_aug.rearrange("p a b c -> p (a b) c")[:, :, 64:65], 1.0
            )
            # per-head S inputs (fp8): qt = [hi|lo] in DR j-dim; kt single
            qts = [qkpool.tile([128, 2, T], FP8, name=f"qt{h}") for h in range(HPC)]
            kts = [qkpool.tile([128, T], FP8, name=f"kt{h}") for h in range(HPC)]
            ktlos = [qkpool.tile([128, T], FP8, name=f"ktlo{h}") for h in range(HPC)]
            y_t = persist.tile([128, NTT, HPC, 64], BF16)
            yT = persist.tile([128, 2, T], BF16)
            w2 = persist.tile([128, 2, 1024], BF16)
            
            from concourse.tile_rust import add_dep_helper
            act_groups = {"E": [], "L": []}
            # ---------------- Phase A: QKV projection ----------------
            mupool_ctx = tc.tile_pool(name="mupool", bufs=1)
            mupool = mupool_ctx.__enter__()
            mu = [mupool.tile([128, T], BF16, name=f"mu{h}") for h in range(HPC)]

            a5_ctx = tc.tile_pool(name="a5", bufs=2)
            a5 = a5_ctx.__enter__()
            abp_ctx = tc.tile_pool(name="abp", bufs=2)
            abp = abp_ctx.__enter__()
            phb_ctx = tc.tile_pool(name="phB", bufs=17)
            phb = phb_ctx.__enter__()
            ost_ctx = tc.tile_pool(name="ostage", bufs=3)
            ostage = ost_ctx.__enter__()
            rn_ctx = tc.tile_pool(name="rn", bufs=2)
            rnp = rn_ctx.__enter__()
            pha_ctx = tc.tile_pool(name="phA", bufs=1)
            pha = pha_ctx.__enter__()
            xhi = pha.tile([128, 4, 2, T], FP8)
            xlo = pha.tile([128, 4, 2, T], FP8)
            wqk = pha.tile([128, 4, 2, 512], FP8)
            wvh = pha.tile([128, 4, 2, 256], FP8)
            wvl = pha.tile([128, 4, 2, 256], FP8)
            nc.sync.dma_start(wqk, wqk_d.rearrange("o p j e -> p o j e"))
            engs = [nc.sync, nc.gpsimd, nc.sync, nc.gpsimd]
            for th in range(2):
                tsl = slice(th * 1024, th * 1024 + 1024)
                for cc in range(4):
                    engs[cc].dma_start(xhi[:, cc, :, tsl], xhi_d[cc][:, :, tsl])
            nc.sync.dma_start(wvh, wvh_d.rearrange("o p j e -> p o j e"))
            nc.sync.dma_start(wvl, wvl_d.rearrange("o p j e -> p o j e"))
            for th in range(2):
                tsl = slice(th * 1024, th * 1024 + 1024)
                for cc in range(4):
                    engs[(cc + 1) % 4].dma_start(
                        xlo[:, cc, :, tsl], xlo_d[cc][:, :, tsl]
                    )
            nc.gpsimd.dma_start(trig, trig_d)
            nc.gpsimd.dma_start(mlow, mlow_d)
            nc.gpsimd.dma_start(mful, mful_d)
            nc.gpsimd.dma_start(idsp, idsp_d)
            nc.gpsimd.dma_start(i128, i128_d)
            nc.gpsimd.dma_start(w2, w2_d.rearrange("o p e -> p o e"))

            # qk: per (head, 1024-block): psum [128,1024], 4 DR matmuls over cc
            # v: out[t, 256] per t-tile, 4 tiles per psum; 3-term fp8 DR.
            # Interleave qk and v allocations so PE has v work while ACT
            # drains softplus.
            def emit_qk(h, tb):
                ts_ = slice(tb * 1024, tb * 1024 + 1024)
                qk_ps = ps2.tile([128, 1024], F32, tag="s2", name="qk_ps")
                for half in range(2):
                    hs = slice(tb * 1024 + half * 512, tb * 1024 + half * 512 + 512)
                    for cc in range(4):
                        nc.tensor.matmul(
                            qk_ps[:, half * 512 : half * 512 + 512],
                            lhsT=wqk[:, cc, :, h * 128 : (h + 1) * 128],
                            rhs=xhi[:, cc, :, hs],
                            start=(cc == 0),
                            stop=(cc == 3),
                            perf_mode=PM.DoubleRow,
                        )
                # softplus = ln(exp(q/64)+1): exp -> mu (bf16), ln batched
                # later so the ACT table doesn't ping-pong between sets
                act_groups["E"].append(
                    nc.scalar.activation(mu[h][:, ts_], qk_ps, AF.Exp, scale=c_inv64)
                )

            def emit_v(tq):  # tq = t-pair index 0..7
                v_ps = psc.tile([128, 2, 256], F32, tag="pc", name="v_ps")
                for s in range(2):
                    tt = 2 * tq + s
                    tsl = slice(tt * 128, (tt + 1) * 128)
                    terms = ((xhi, wvh), (xhi, wvl), (xlo, wvh))
                    n = 0
                    for cc in range(4):
                        for (xx, ww) in terms:
                            nc.tensor.matmul(
                                v_ps[:, s, :],
                                lhsT=xx[:, cc, :, tsl],
                                rhs=ww[:, cc],
                                start=(n == 0),
                                stop=(n == 11),
                                perf_mode=PM.DoubleRow,
                            )
                            n += 1
                nc.vector.tensor_copy(
                    out=v_aug[:, 2 * tq : 2 * tq + 2, :, 0:64],
                    in_=v_ps.rearrange("p s (h e) -> p s h e", h=HPC),
                )

            # ------- Phase A.5: Qt/Kt formation (per head) -------
            def emit_a5(h):
                abh = abp.tile([128, T], BF16, tag="ab", name="abh")
                nc.sync.dma_start(abh, ab_d[h])
                # musw = [mu_k ; mu_q] (swapped halves) via DVE shuffles;
                # processed in 1024-col halves, high half first (jb runs
                # descending, so high-t qt cols are needed first; kt pair 0
                # needs low-t keys first -> kt low half first)
                musw = a5.tile([128, T], BF16, tag="musw", name="musw")
                qtf = a5.tile([128, T], BF16, tag="qtf", name="qtf")
                ktf = a5.tile([128, T], BF16, tag="ktf", name="ktf")
                idm = list(range(32))
                qeng = nc.vector if h == 0 else nc.gpsimd
                for hb in (0,) if h == 0 else ((1, 0)):
                    # kt chain on Pool (low keys first: S pair 0 reads them)
                    ts_ = slice(hb * 1024, hb * 1024 + 1024)
                    if h == 0:
                        nc.vector.stream_shuffle(
                            musw[0:64, ts_], mu[h][64:128, ts_], idm
                        )
                    else:
                        nc.sync.dma_start(musw[0:64, ts_], mu[h][64:128, ts_])
                    nc.gpsimd.tensor_mul(
                        ktf[0:64, ts_], musw[0:64, ts_], abh[0:64, ts_]
                    )
                    nc.gpsimd.tensor_mul(
                        ktf[64:128, ts_], mu[h][64:128, ts_], abh[64:128, ts_]
                    )
                    nc.gpsimd.tensor_copy(out=kts[h][:, ts_], in_=ktf[:, ts_])
                    nc.gpsimd.tensor_sub(
                        ktlos[h][:, ts_], ktf[:, ts_], kts[h][:, ts_]
                    )
                for hb in ((1, 0) if h != 0 else (1,)):
                    # qt chain on DVE (h0: incl. fp8 ops, parallel with Pool)
                    ts_ = slice(hb * 1024, hb * 1024 + 1024)
                    if h == 0:
                        nc.vector.stream_shuffle(
                            musw[64:128, ts_], mu[h][0:64, ts_], idm
                        )
                    else:
                        nc.sync.dma_start(musw[64:128, ts_], mu[h][0:64, ts_])
                    nc.vector.tensor_mul(
                        qtf[0:64, ts_], mu[h][0:64, ts_], trig[0:64, ts_]
                    )
                    nc.vector.tensor_mul(
                        qtf[64:128, ts_], musw[64:128, ts_], trig[64:128, ts_]
                    )
                    qeng.tensor_copy(
                        out=qts[h][:, 0, ts_], in_=qtf[:, ts_]
                    )
                    qeng.tensor_sub(
                        qts[h][:, 1, ts_], qtf[:, ts_], qts[h][:, 0, ts_]
                    )
                if h == 0:
                    for ts_ in (slice(1024, 2048), slice(0, 1024)):
                        nc.vector.stream_shuffle(
                            musw[0:64, ts_] if ts_.start == 1024 else musw[64:128, ts_],
                            mu[h][64:128, ts_] if ts_.start == 1024 else mu[h][0:64, ts_],
                            idm,
                        )
                    ts_ = slice(1024, 2048)
                    nc.gpsimd.tensor_mul(
                        ktf[0:64, ts_], musw[0:64, ts_], abh[0:64, ts_]
                    )
                    nc.gpsimd.tensor_mul(
                        ktf[64:128, ts_], mu[h][64:128, ts_], abh[64:128, ts_]
                    )
                    nc.gpsimd.tensor_copy(out=kts[h][:, ts_], in_=ktf[:, ts_])
                    nc.gpsimd.tensor_sub(
                        ktlos[h][:, ts_], ktf[:, ts_], kts[h][:, ts_]
                    )
                    ts_ = slice(0, 1024)
                    nc.vector.tensor_mul(
                        qtf[0:64, ts_], mu[h][0:64, ts_], trig[0:64, ts_]
                    )
                    nc.vector.tensor_mul(
                        qtf[64:128, ts_], musw[64:128, ts_], trig[64:128, ts_]
                    )
                    nc.vector.tensor_copy(out=qts[h][:, 0, ts_], in_=qtf[:, ts_])
                    nc.vector.tensor_sub(
                        qts[h][:, 1, ts_], qtf[:, ts_], qts[h][:, 0, ts_]
                    )

            for h in range(HPC):
                emit_qk(h, 0)
                emit_qk(h, 1)
            for h in range(HPC):
                ln = nc.scalar.activation(mu[h], mu[h], AF.Ln, bias=1.0)
                add_dep_helper(ln.ins, act_groups["E"][-1].ins, sync=False,
                               reason="group phase-A Lns after Exps (ACT table)")
                act_groups["L"].append(ln)
            emit_a5(0)
            for tq in range(8):
                emit_v(tq)

            pha_ctx.__exit__(None, None, None)

            for h in range(1, HPC):
                emit_a5(h)

            # ------- Phase B: attention + transpose + c_proj, per tq block ----
            proj_pending = []
            emit_proj_fns = {}
            pending = None
            for jb in (3, 2, 1, 0):
                cols = slice(jb * 512, (jb + 1) * 512)
                def emit_y(h, p_sbs, y_ps, yjb=None):
                    yjb = jb if yjb is None else yjb
                    for q in range(4):
                        jt = 4 * yjb + q
                        for kt in range(jt + 1):
                            nc.tensor.matmul(
                                y_ps[:, q, :],
                                lhsT=p_sbs[kt // 2][:, kt % 2, q * 128 : (q + 1) * 128],
                                rhs=v_aug[:, kt, h, :],
                                start=(kt == 0),
                                stop=(kt == jt),
                            )
                    rn = rnp.tile([128, 4], F32, tag="rn", name="rn")
                    nc.vector.reciprocal(
                        rn, y_ps[:, :, 64:65].rearrange("p a b -> p (a b)")
                    )
                    rnb = rn.rearrange("p (a b) -> p a b", b=1).broadcast_to(
                        [128, 4, 64]
                    )
                    nc.vector.tensor_mul(
                        y_t[:, 4 * yjb : 4 * yjb + 4, h, :], y_ps[:, :, 0:64], rnb
                    )

                for h in range(HPC):
                    y_ps = psy.tile([128, 4, 65], F32, tag="y1", name="y_ps")
                    p_sbs = []
                    for p in range(2 * jb + 2):
                        c0 = max(0, 256 * p - 512 * jb)
                        csl = slice(jb * 512 + c0, (jb + 1) * 512)
                        sps = ps2.tile([128, 2, 512], F32, tag="s2", name="sps")
                        for s in range(2):
                            kt = 2 * p + s
                            ksl = slice(kt * 128, (kt + 1) * 128)
                            ktb = kts[h][:, ksl].rearrange(
                                "p (j m) -> p j m", j=1
                            ).broadcast_to([128, 2, 128])
                            ktlob = ktlos[h][:, ksl].rearrange(
                                "p (j m) -> p j m", j=1
                            ).broadcast_to([128, 2, 128])
                            nc.tensor.matmul(
                                sps[:, s, c0:512],
                                lhsT=ktb,
                                rhs=qts[h][:, :, csl],
                                start=True,
                                stop=False,
                                perf_mode=PM.DoubleRow,
                            )
                            d0 = 128 * kt - 512 * jb  # diag block col offset
                            if 0 <= d0 < 512:
                                nc.tensor.matmul(
                                    sps[:, s, d0 : d0 + 128],
                                    lhsT=mlow,
                                    rhs=idsp,
                                    start=False,
                                    stop=False,
                                    perf_mode=PM.DoubleRow,
                                )
                                if d0 > c0:
                                    # fully-masked strip left of the diagonal
                                    nc.tensor.matmul(
                                        sps[:, s, c0:d0],
                                        lhsT=mful,
                                        rhs=idsp[:, :, 0 : d0 - c0],
                                        start=False,
                                        stop=False,
                                        perf_mode=PM.DoubleRow,
                                    )
                            nc.tensor.matmul(
                                sps[:, s, c0:512],
                                lhsT=ktlob,
                                rhs=qts[h][:, :, csl],
                                start=False,
                                stop=True,
                                perf_mode=PM.DoubleRow,
                            )
                        p_sb = phb.tile([128, 2, 512], BF16, tag="p", name="p_sb")
                        ex = nc.scalar.activation(
                            p_sb[:, :, c0:512],
                            sps[:, :, c0:512],
                            AF.Exp,
                            scale=c_inv8,
                            bias=c_bias,
                        )
                        if act_groups["L"]:
                            add_dep_helper(ex.ins, act_groups["L"][-1].ins, sync=False,
                                           reason="phase-B exps after phase-A Lns (ACT table)")
                            act_groups["L"] = []
                        p_sbs.append(p_sb)
                    if pending is not None:
                        emit_y(*pending)
                        pending = None
                    if h == 0 and proj_pending:
                        emit_proj_fns[proj_pending.pop()]()
                    pending = (h, p_sbs, y_ps, jb)
                # transpose 4 t-tiles + c_proj + stage + dma out (deferred:
                # emitted after the next jb's first head S/exp so PE's
                # in-order queue doesn't stall ACT at jb boundaries)
                def emit_proj(jb=jb, last=False):
                  for q in range(4):
                    tt = 4 * jb + q
                    tsl = slice(tt * 128, (tt + 1) * 128)
                    pc = psc.tile([128, 2, 128], BF16, tag="pc", name="pc")
                    for cc in range(2):
                        nc.tensor.matmul(
                            pc[:, cc, :],
                            lhsT=y_t[:, tt, 2 * cc : 2 * cc + 2, :],
                            rhs=i128,
                            start=True,
                            stop=True,
                            is_transpose=True,
                        )
                    nc.vector.tensor_copy(out=yT[:, :, tsl], in_=pc)
                  for q in range(4):
                    tt = 4 * jb + q
                    tsl = slice(tt * 128, (tt + 1) * 128)
                    ost = ostage.tile([128, 1024], F32, tag="o", name="ost")
                    for eh in range(2):
                        po = psc.tile([128, 512], F32, tag="pc", name="po")
                        for cc in range(2):
                            nc.tensor.matmul(
                                po,
                                lhsT=yT[:, cc, tsl],
                                rhs=w2[:, cc, eh * 512 : (eh + 1) * 512],
                                start=(cc == 0),
                                stop=(cc == 1),
                            )
                        if last and (2 * q + eh) % 2 == 1:
                            nc.scalar.copy(ost[:, eh * 512 : (eh + 1) * 512], po)
                        else:
                            nc.vector.tensor_copy(
                                out=ost[:, eh * 512 : (eh + 1) * 512], in_=po
                            )
                        if last:
                            deng = nc.sync if (2 * q + eh) % 2 == 0 else nc.gpsimd
                            deng.dma_start(
                                out_d[tt][:, eh * 512 : (eh + 1) * 512],
                                ost[:, eh * 512 : (eh + 1) * 512],
                            )
                    if not last:
                        deng = nc.sync if q % 2 == 0 else nc.gpsimd
                        deng.dma_start(out_d[tt], ost)
                emit_proj_fns[jb] = emit_proj
                proj_pending.append(jb)
            if pending is not None:
                emit_y(*pending)
            while proj_pending:
                emit_proj_fns[proj_pending.pop()](last=True)
            rn_ctx.__exit__(None, None, None)
            ost_ctx.__exit__(None, None, None)
            phb_ctx.__exit__(None, None, None)
            abp_ctx.__exit__(None, None, None)
            a5_ctx.__exit__(None, None, None)
            mupool_ctx.__exit__(None, None, None)

    nc.compile()
    return nc


def make_inputs(x, w_attn, w_proj, delta):
    """Host-side prep: per-core input dicts (core = b*4 + g)."""
    x = np.asarray(x, dtype=np.float32)
    w_attn = np.asarray(w_attn, dtype=np.float32)
    w_proj = np.asarray(w_proj, dtype=np.float32)
    delta = np.asarray(delta, dtype=np.float32)

    inv_freq = 1.0 / (BASE ** (np.arange(D, dtype=np.float32) / D))
    t = np.arange(T, dtype=np.float64)
    freqs = (t[:, None] * inv_freq[None, :].astype(np.float64)).astype(np.float32)
    cosT = np.cos(freqs).T.astype(np.float32)  # (D, T)
    sinT = np.sin(freqs).T.astype(np.float32)
    trig = np.concatenate([cosT, sinT], axis=0).astype(ml_dtypes.bfloat16)
    d = np.clip(delta, -2.0 * math.pi, 0.0)

    # fp8 split of x per batch: [4cc, 128p, 2j, T] with c = cc*256+j*128+p
    def to_dr(mat):  # mat (T, C) -> (4, 128, 2, T)
        m = mat.T.reshape(4, 2, 128, T)  # (cc, j, p, t)
        return np.ascontiguousarray(m.transpose(0, 2, 1, 3))

    xw = [None] * B
    for b in range(B):
        xb = x[b]  # (T, C)
        x_hi = xb.astype(E4)
        x_lo = (xb - x_hi.astype(np.float32)).astype(E4)
        xw[b] = (to_dr(x_hi.astype(np.float32)).astype(E4),
                 to_dr(x_lo.astype(np.float32)).astype(E4))

    qw = w_attn[:C].reshape(H, D, C)
    kw = w_attn[C : 2 * C].reshape(H, D, C)
    vw = w_attn[2 * C :].reshape(H, D, C)

    # mask constants
    pp, jj, tk = np.meshgrid(
        np.arange(64), np.arange(2), np.arange(128), indexing="ij"
    )
    f = jj * 64 + pp
    mlow = np.where(tk > f, -240.0, 0.0).astype(E4)
    mful = np.full((64, 2, 128), -240.0, dtype=np.float32).astype(E4)
    idsp = (tk == f).astype(np.float32).astype(E4)
    i128 = np.eye(128, dtype=np.float32).astype(ml_dtypes.bfloat16)

    in_maps = []
    for core in range(N_CORES):
        b, g = divmod(core, HPC)
        heads = list(range(HPC * g, HPC * g + HPC))

        # wqk: (4cc, 128p, 2j, 512): col = h*128 + r; r<64 q_d else k_d
        wqk_full = np.empty((C, 512), dtype=np.float32)  # (c, col)
        for hi_, hg in enumerate(heads):
            wqk_full[:, hi_ * 128 : hi_ * 128 + 64] = qw[hg].T * WSCALE
            wqk_full[:, hi_ * 128 + 64 : hi_ * 128 + 128] = kw[hg].T * WSCALE
        wqk8 = wqk_full.astype(E4)
        wqk_dr = np.ascontiguousarray(
            wqk8.reshape(4, 2, 128, 512).transpose(0, 2, 1, 3)
        )

        wv_full = (
            vw[HPC * g : HPC * g + HPC].reshape(256, C).T * WSCALE
        )  # (c, 256)
        wv_hi = wv_full.astype(E4)
        wv_lo = (wv_full - wv_hi.astype(np.float32)).astype(E4)
        wvh_dr = np.ascontiguousarray(
            wv_hi.reshape(4, 2, 128, 256).transpose(0, 2, 1, 3)
        )
        wvl_dr = np.ascontiguousarray(
            wv_lo.reshape(4, 2, 128, 256).transpose(0, 2, 1, 3)
        )

        ab = np.stack(
            [
                np.concatenate(
                    [
                        np.cos(freqs + d[hg][None, :]).T,
                        np.sin(freqs + d[hg][None, :]).T,
                    ],
                    axis=0,
                ).astype(ml_dtypes.bfloat16)
                for hg in heads
            ],
            axis=0,
        )  # (4, 128, T)

        # w2: (2cc, 128p, 1024e): channel c_local = cc*128 + p of this group's
        # 256 y channels; y channel (h_local, dd) flattened h_local*64+dd
        w2g = w_proj[:, 256 * g : 256 * (g + 1)]  # (e, 256)
        w2_dr = np.ascontiguousarray((w2g.T / WSCALE).reshape(2, 128, 1024)).astype(ml_dtypes.bfloat16)

        in_maps.append(
            {
                "xhi": xw[b][0],
                "xlo": xw[b][1],
                "wqk": wqk_dr,
                "wvh": wvh_dr,
                "wvl": wvl_dr,
                "trig": trig,
                "ab": ab,
                "mlow": mlow,
                "mful": mful,
                "idsp": idsp,
                "i128": i128,
                "w2": w2_dr,
            }
        )
    return in_maps


_NC_CACHE = []


def _get_nc():
    if not _NC_CACHE:
        _NC_CACHE.append(build_module())
    return _NC_CACHE[0]


def kernel(x, w_attn, w_proj, delta, _trace=False):
    in_maps = make_inputs(x, w_attn, w_proj, delta)
    nc = _get_nc()
    res = None
    outs = None
    last_err = None
    for attempt in range(3):
        try:
            res = bass_utils.run_bass_kernel_spmd(
                nc, in_maps, core_ids=list(range(N_CORES)), trace=_trace
            )
            outs = [np.asarray(r["out"]).reshape(T, C) for r in res.results]
            break
        except Exception as e:
            last_err = e
            if "unrecoverable" not in str(e).lower() or attempt == 2:
                raise
            import time as _time

            _time.sleep(2.0)
    assert outs is not None, last_err
    if _trace:
        kernel.last_results = res
    full = np.zeros((B, T, C), dtype=np.float32)
    for core in range(N_CORES):
        full[core // HPC] += outs[core]
    return full
